# revision 6
# baseline (speedup 1.0000x reference)
"""Trainium2 Bass kernel for nn_Attention_40346922778795 (v2 schedule).

8 layers of: conv3x3+ReLU -> GAP -> DSU recurrence; k/v modulation driven by
batch-0 hidden state; additive attention over layer history.

Distribution: batch data-parallel, 4 owned batches + 1 replicated batch-0
lane per core (hidden[0][0] dependency stays local; zero collectives).

v2 schedule changes vs v1 baseline (301us):
 - conv activations write straight into the next xp buffer; the attention
   context accumulates IN-PLACE there (no xc staging tile).
 - bf16 feature maps by default (KF32=1 env falls back to fp32r): DVE AXPYs
   run in 2x packed mode, LDWEIGHTS halves, DMA halves.
 - softmax exp computed as sigmoid ratio e^s = p/(1-p), keeping ScalarE in
   the sigmoid activation table all kernel long (no ACT_TABLE_LOAD thrash).
 - DSU / cell_hh / kv-gate matmul chains interleaved INTO the conv matmul
   stream so TensorE never idles waiting on them and stays at high p-state.
 - per-lane context routing: lane0 on TensorE (diag matmuls, feeds next
   layer's first conv ASAP), lanes 1-2 DVE inline, lanes 3-4 DVE deferred
   into the next layer's stream (their convs run late, so there is slack).
 - last layer: lane-0 context + store skipped (replica lane is dead there),
   lane4 context on TensorE to shorten the DVE drain.
"""

import os
import sys

for _p in ("/opt/trn_rl_repo",):
    if _p not in sys.path and os.path.isdir(_p):
        sys.path.insert(0, _p)
os.environ.setdefault("MYCRO_LOCAL_CACHE", "1")

import numpy as np  # noqa: E402

B, C, H, W, L = 32, 128, 28, 28, 8
R = C // 4
BL = 5          # lanes per core: [batch0 replica, 4 owned batches]
KF32 = bool(os.environ.get("KF32"))
HP = H + 2
# bf16 x-buffers use left-pad 2 so every 28-wide interior row starts on a
# 4-byte boundary (DVE 2x packed mode requirement); fp32r keeps pad 1.
WL = 1 if KF32 else 2
WP = 30 if KF32 else 32
NCORES = 8
CH = 14         # conv chunk rows
BLP = 6         # lane count padded to even (fp32r matmul ISA restriction)
NCHUNK = H // CH


def build_nc(n_layers=L):
    import concourse.bacc as bacc
    import concourse.mybir as mybir
    import concourse.tile as tile

    dt = mybir.dt
    AF = mybir.ActivationFunctionType
    OP = mybir.AluOpType
    f32, f32r = dt.float32, dt.float32r
    xdt = f32r if KF32 else dt.bfloat16

    nc = bacc.Bacc()

    X0 = nc.dram_tensor("x0", [C, BL, HP, WP], xdt, kind="ExternalInput")
    CW = nc.dram_tensor("cw", [C, L, 9, C], xdt, kind="ExternalInput")
    CB = nc.dram_tensor("cb", [C, L], f32, kind="ExternalInput")
    W1 = nc.dram_tensor("w1", [C, 2, R], f32r, kind="ExternalInput")
    B1 = nc.dram_tensor("b1", [R, 2], f32, kind="ExternalInput")
    W2 = nc.dram_tensor("w2", [R, 2, 3, C], f32r, kind="ExternalInput")
    # b2t columns: [i, f, 2*c] (c-gate bias pre-doubled for the tanh trick)
    B2T = nc.dram_tensor("b2t", [C, 3], f32, kind="ExternalInput")
    ONES = nc.dram_tensor("ones", [C, C], f32r, kind="ExternalInput")
    EYE = nc.dram_tensor("eye", [C, C], xdt, kind="ExternalInput")
    OUT = nc.dram_tensor("out", [C, BL, HP, WP], xdt, kind="ExternalOutput")

    inv_hw = 1.0 / (H * W)
    exp_scale = 1.0 / (float(H * W) ** 2 * np.sqrt(np.float32(C)))
    KV = 2 * L * BL   # width of the merged k/v modulation block

    with tile.TileContext(nc) as tc:
        with tc.tile_pool(name="xpad", bufs=1) as xpool, \
             tc.tile_pool(name="wts", bufs=2) as wpool, \
             tc.tile_pool(name="small", bufs=1) as spool, \
             tc.tile_pool(name="work", bufs=2) as wk, \
             tc.tile_pool(name="dpool", bufs=8) as dpool, \
             tc.tile_pool(name="pc", bufs=3, space="PSUM") as pc, \
             tc.tile_pool(name="pg", bufs=3, space="PSUM") as pg, \
             tc.tile_pool(name="pu", bufs=1, space="PSUM") as pu:

            # ---- persistent state (attention tensors are [C, BL, L]) ----
            xp = [xpool.tile([C, BL, HP, WP], xdt, tag=f"xp{i}", name=f"xp{i}")
                  for i in range(L)]
            xp.append(xp[0])  # layer-8 output reuses the input buffer
            kr = spool.tile([C, BL, L], f32r, tag="kr")     # raw q sums, modulated
            mm = spool.tile([C, BL, L], f32, tag="mm")      # running v modulation
            pr = spool.tile([C, BL, L], f32, tag="pr")      # raw pooled-v sums
            w1sb = spool.tile([C, 2, R], f32r, tag="w1sb")
            b1sb = spool.tile([R, 2], f32, tag="b1sb")
            w2sb = spool.tile([R, 2, 3, C], f32r, tag="w2sb")
            b2tsb = spool.tile([C, 3], f32, tag="b2t")
            cbsb = spool.tile([C, L], f32, tag="cbsb")
            ones = spool.tile([C, C], f32r, tag="ones")
            eye = spool.tile([C, C], xdt, tag="eye")
            ht = spool.tile([C, BLP], f32r, tag="ht")
            ct = spool.tile([C, BL], f32, tag="ct")
            ht0 = spool.tile([C, 2], f32r, tag="ht0")
            ct0 = spool.tile([C, 1], f32, tag="ct0")
            seq = spool.tile([C, BLP], f32r, tag="seq")
            seq0 = spool.tile([C, 2], f32r, tag="seq0")
            kvin = spool.tile([C, KV], f32r, tag="kvin")
            qk = spool.tile([C, BL, L], f32r, tag="qk")
            sig = spool.tile([C, BL, L], f32, tag="sig")
            om = spool.tile([C, BL, L], f32, tag="om")
            rr = spool.tile([C, BL, L], f32, tag="rr")
            ee = spool.tile([C, BL, L], f32, tag="ee")
            den = spool.tile([C, BL], f32, tag="den")
            dinv = spool.tile([C, BL], f32, tag="dinv")
            aw = spool.tile([C, BL, L], f32, tag="aw")
            awh = spool.tile([C, BL, L], xdt, tag="awh")
            gsum = spool.tile([C, BL, NCHUNK], f32, tag="gsum")
            pacc = spool.tile([C, BL, NCHUNK], f32, tag="pacc")

            # ---- one-time loads & init ----
            wsb0 = wpool.tile([C, 9, C], xdt, tag="wsb", name="wsb0")
            nc.sync.dma_start(wsb0[:], CW[:, 0])
            for b in range(BL):
                nc.sync.dma_start(xp[0][:, b], X0[:, b])
            nc.sync.dma_start(w1sb[:], W1[:])
            nc.sync.dma_start(b1sb[:], B1[:])
            nc.sync.dma_start(w2sb[:], W2[:])
            nc.sync.dma_start(b2tsb[:], B2T[:])
            nc.sync.dma_start(cbsb[:], CB[:])
            nc.sync.dma_start(ones[:], ONES[:])
            nc.sync.dma_start(eye[:], EYE[:])
            nc.gpsimd.memset(ht[:].bitcast(f32), 0.0)
            nc.gpsimd.memset(ct[:], 0.0)
            nc.gpsimd.memset(ht0[:].bitcast(f32), 0.0)
            nc.gpsimd.memset(ct0[:], 0.0)
            nc.gpsimd.memset(seq[:].bitcast(f32), 0.0)
            nc.gpsimd.memset(seq0[:].bitcast(f32), 0.0)
            nc.gpsimd.memset(kvin[:].bitcast(f32), 0.0)
            nc.gpsimd.memset(kr[:].bitcast(f32), 0.0)
            nc.gpsimd.memset(qk[:].bitcast(f32), 0.0)
            nc.gpsimd.memset(mm[:], 0.0)
            nc.gpsimd.memset(pr[:], 0.0)
            # pad borders of buffers 1..7 are zeroed lazily, one buffer per
            # layer, inside emit_layer (keeps Pool off the startup critical
            # path; buffer 8 aliases the host-zero-padded input buffer)
            bdt = f32 if KF32 else dt.uint16

            def zero_borders(i):
                nc.gpsimd.memset(xp[i][:, :, 0, :].bitcast(bdt), 0)
                nc.gpsimd.memset(xp[i][:, :, HP - 1, :].bitcast(bdt), 0)
                nc.gpsimd.memset(xp[i][:, :, 1:HP - 1, 0:WL].bitcast(bdt), 0)
                nc.gpsimd.memset(xp[i][:, :, 1:HP - 1, WL + W:WP].bitcast(bdt), 0)
            zero_borders(1)

            def conv_mm(idx, b, ch, wsb):
                h0 = ch * CH
                ps = pc.tile([C, CH, W], f32, tag="c", name="ps")
                for tap in range(9):
                    dy, dx = tap // 3, tap % 3
                    rhs = xp[idx][:, b, h0 + dy:h0 + dy + CH,
                                  dx + WL - 1:dx + WL - 1 + W]
                    nc.tensor.matmul(ps[:], wsb[:, tap, :], rhs,
                                     start=(tap == 0), stop=(tap == 8))
                return ps

            def conv_act(idx, b, ch, ps):
                h0 = ch * CH
                dst = xp[idx + 1][:, b, 1 + h0:1 + h0 + CH, WL:WL + W]
                nc.scalar.activation(dst, ps[:], AF.Relu,
                                     bias=cbsb[:, idx:idx + 1], scale=1.0,
                                     accum_out=gsum[:, b, ch:ch + 1])

            def seq_add(b):
                dst = seq0[:, 0:1] if b == 0 else seq[:, b:b + 1]
                with nc.allow_low_precision("f32r rounding of f32 sum"):
                    nc.vector.tensor_tensor(dst, gsum[:, b, 0:1].bitcast(f32r),
                                            gsum[:, b, 1:2].bitcast(f32r),
                                            op=OP.add)

            def gates_post(pgt, nw, bias_i, bias_f, bias_c2, name):
                """sigmoid(i), sigmoid(f), sigmoid(2c+2bc) on [C, nw]."""
                si = wk.tile([C, max(nw, BL)], f32, tag=f"si{name}", name="si")
                sf = wk.tile([C, max(nw, BL)], f32, tag=f"sf{name}", name="sf")
                s2 = wk.tile([C, max(nw, BL)], f32, tag=f"s2{name}", name="s2")
                nc.scalar.activation(si[:, :nw], pgt[0][:, :nw], AF.Sigmoid,
                                     bias=bias_i)
                nc.scalar.activation(sf[:, :nw], pgt[1][:, :nw], AF.Sigmoid,
                                     bias=bias_f)
                nc.scalar.activation(s2[:, :nw], pgt[2][:, :nw], AF.Sigmoid,
                                     bias=bias_c2, scale=2.0)
                return si, sf, s2

            def gate_combine(si, sf, s2, cx_ap, nw, name):
                """ncx = sf*cx + si*(2*s2-1); returns [C, nw] f32 tile."""
                tmp = wk.tile([C, max(nw, BL)], f32, tag=f"tm{name}", name="tmp")
                nc.vector.scalar_tensor_tensor(tmp[:, :nw], s2[:, :nw], 2.0,
                                               si[:, :nw], op0=OP.mult,
                                               op1=OP.mult)
                ncx = wk.tile([C, max(nw, BL)], f32, tag=f"nx{name}", name="ncx")
                if cx_ap.free_size() == 1:
                    nc.vector.scalar_tensor_tensor(ncx[:, :nw], sf[:, :nw],
                                                   cx_ap, tmp[:, :nw],
                                                   op0=OP.mult, op1=OP.add)
                else:
                    t2 = wk.tile([C, max(nw, BL)], f32, tag=f"t2{name}",
                                 name="t2")
                    nc.vector.tensor_tensor(t2[:, :nw], sf[:, :nw], cx_ap,
                                            op=OP.mult)
                    nc.vector.tensor_tensor(ncx[:, :nw], t2[:, :nw], tmp[:, :nw],
                                            op=OP.add)
                nc.vector.tensor_tensor(ncx[:, :nw], ncx[:, :nw], si[:, :nw],
                                        op=OP.subtract)
                return ncx

            def dsu_update(sq, lo, hi, nw):
                """DSU over input columns [lo:hi) (hi-lo even); writes the
                first nw lanes' ht/ct columns [lo:lo+nw)."""
                w_ = hi - lo
                p1 = pu.tile([R, KV], f32, tag="u0", name="p1")
                nc.tensor.matmul(p1[:, :w_], w1sb[:, 0, :], sq[:, lo:hi],
                                 start=True, stop=True)
                p2 = pu.tile([R, KV], f32, tag="u1", name="p2")
                nc.tensor.matmul(p2[:, :w_], w1sb[:, 1, :], ht[:, lo:hi],
                                 start=True, stop=True)
                u1i = wk.tile([R, BLP], f32r, tag="u1i", name="u1i")
                nc.scalar.activation(u1i[:, :w_], p1[:, :w_], AF.Relu,
                                     bias=b1sb[:, 0:1], scale=inv_hw)
                u1h = wk.tile([R, BLP], f32r, tag="u1h", name="u1h")
                nc.scalar.activation(u1h[:, :w_], p2[:, :w_], AF.Relu,
                                     bias=b1sb[:, 1:2], scale=1.0)
                pgt = []
                for g in range(3):
                    pge = pg.tile([C, KV], f32, tag="g", name="pge")
                    nc.tensor.matmul(pge[:, :w_], w2sb[:, 0, g, :], u1i[:, :w_],
                                     start=True, stop=False)
                    nc.tensor.matmul(pge[:, :w_], w2sb[:, 1, g, :], u1h[:, :w_],
                                     start=False, stop=True)
                    pgt.append(pge)
                si, sf, s2 = gates_post(pgt, nw, b2tsb[:, 0:1],
                                        b2tsb[:, 1:2], b2tsb[:, 2:3], "x")
                ncx = gate_combine(si, sf, s2, ct[:, lo:lo + nw], nw, "x")
                nc.vector.tensor_copy(ct[:, lo:lo + nw], ncx[:, :nw])
                nc.scalar.activation(ht[:, lo:lo + nw], ct[:, lo:lo + nw],
                                     AF.Sigmoid, bias=0.0, scale=1.0)

            def softmax_group(idx, s0, s1):
                """qk -> scores -> sigmoid-ratio softmax -> aw for lanes
                [s0, s1)."""
                t, tp = idx, idx + (idx % 2)
                w_ = s1 - s0
                for b in range(s0, s1):
                    sqcol = seq0[:, 0:1] if b == 0 else seq[:, b:b + 1]
                    nc.vector.tensor_tensor(qk[:, b, :t], kr[:, b, :t],
                                            sqcol.broadcast_to([C, t]),
                                            op=OP.mult)
                psc = pg.tile([C, KV], f32, tag="g", name="psc")
                p3 = psc[:, :w_ * tp].rearrange("p (b t) -> p b t", b=w_)
                nc.tensor.matmul(p3[:], ones[:], qk[:, s0:s1, 0:tp],
                                 start=True, stop=True)
                nc.scalar.activation(sig[:, s0:s1, :t], p3[:, :, :t],
                                     AF.Sigmoid, bias=0.0, scale=exp_scale)
                one3 = ones[:, 0:w_ * t].bitcast(f32).rearrange(
                    "p (b t) -> p b t", b=w_)
                nc.vector.scalar_tensor_tensor(om[:, s0:s1, :t],
                                               sig[:, s0:s1, :t], -1.0,
                                               one3, op0=OP.mult, op1=OP.add)
                nc.vector.reciprocal(rr[:, s0:s1, :t], om[:, s0:s1, :t])
                nc.vector.tensor_tensor(ee[:, s0:s1, :t], sig[:, s0:s1, :t],
                                        rr[:, s0:s1, :t], op=OP.mult)
                nc.vector.tensor_reduce(den[:, s0:s1], ee[:, s0:s1, :t],
                                        axis=mybir.AxisListType.X, op=OP.add)
                nc.vector.reciprocal(dinv[:, s0:s1], den[:, s0:s1])
                for b in range(s0, s1):
                    nc.vector.tensor_tensor(om[:, b, :t], ee[:, b, :t],
                                            mm[:, b, :t], op=OP.mult)
                    nc.vector.tensor_tensor(
                        aw[:, b, :t], om[:, b, :t],
                        dinv[:, b:b + 1].broadcast_to([C, t]), op=OP.mult)
                with nc.allow_low_precision("bf16 ctx coefficients"):
                    nc.vector.tensor_copy(awh[:, s0:s1, :t], aw[:, s0:s1, :t])

            def ctx_dve_chain(idx, b, accum):
                t = idx
                dst = xp[idx + 1][:, b, 1:1 + H, WL:WL + W]
                for ti in range(t):
                    src = xp[ti + 1][:, b, 1:1 + H, WL:WL + W]
                    kwargs = {}
                    if ti == t - 1 and accum:
                        kwargs["accum_out"] = pr[:, b, idx:idx + 1]
                    nc.vector.scalar_tensor_tensor(dst, src,
                                                   awh[:, b, ti:ti + 1], dst,
                                                   op0=OP.mult, op1=OP.add,
                                                   **kwargs)

            def ctx_diags(idx, b, engines="AAP"):
                t = idx
                dgs = []
                for ti in range(t):
                    dg = dpool.tile([C, C], xdt, tag="diag", name="dg")
                    if engines[ti % len(engines)] == "P":
                        nc.gpsimd.tensor_tensor(
                            dg[:], eye[:],
                            aw[:, b, ti:ti + 1].broadcast_to([C, C]),
                            op=OP.mult)
                    else:
                        nc.scalar.mul(dg[:], eye[:], aw[:, b, ti:ti + 1])
                    dgs.append(dg)
                return dgs

            def ctx_pe_mm(idx, b, dgs, accum):
                t = idx
                for ch in range(NCHUNK):
                    h0 = ch * CH
                    dst = xp[idx + 1][:, b, 1 + h0:1 + h0 + CH, WL:WL + W]
                    ps = pc.tile([C, CH, W], f32, tag="c", name="psx")
                    nc.tensor.matmul(ps[:], eye[:], dst, start=True, stop=False)
                    for k in range(t):
                        nc.tensor.matmul(ps[:], dgs[k][:],
                                         xp[k + 1][:, b, 1 + h0:1 + h0 + CH,
                                                   WL:WL + W],
                                         start=False, stop=(k == t - 1))
                    kwargs = {}
                    if accum:
                        kwargs["accum_out"] = pacc[:, b, ch:ch + 1]
                    nc.scalar.activation(dst, ps[:], AF.Copy, bias=0.0,
                                         scale=1.0, **kwargs)
                if accum:
                    nc.vector.tensor_reduce(
                        pr[:, b, idx:idx + 1],
                        pacc[:, b].rearrange("p (x c) -> p x c", x=1),
                        axis=mybir.AxisListType.X, op=OP.add)

            def emit_layer(idx, wsb, deferred):
                t = idx
                tp = t + (t % 2)
                last = idx == n_layers - 1
                first = idx == 0
                nk = BL * tp
                nkv = 2 * nk

                wsb_next = None
                if idx + 1 < n_layers:
                    wsb_next = wpool.tile([C, 9, C], xdt, tag="wsb",
                                          name="wsbn")
                    nc.sync.dma_start(wsb_next[:], CW[:, idx + 1])
                if 2 + idx < L:
                    zero_borders(idx + 2)   # borders needed by layer idx+1

                # ---- lane 0 conv ----
                conv_act(idx, 0, 0, conv_mm(idx, 0, 0, wsb))
                conv_act(idx, 0, 1, conv_mm(idx, 0, 1, wsb))
                seq_add(0)

                kv_k = kv_v = None
                if not first:
                    # kvin: [ k block | pooled*m block ], lanes 0-2 of the v
                    # block now; lanes 3-4 after the deferred chains below.
                    kv_k = kvin[:, :nk].rearrange("p (b t) -> p b t", b=BL)
                    kv_v = kvin[:, nk:nkv].rearrange("p (b t) -> p b t", b=BL)
                    nc.vector.tensor_copy(kv_k[:, :, :t], kr[:, :, :t])
                    nc.vector.tensor_tensor(kv_v[:, 0:3, :t], pr[:, 0:3, :t],
                                            mm[:, 0:3, :t], op=OP.mult)
                # deferred lane-3/4 context chains from the previous layer
                for fn in deferred:
                    fn()
                deferred = []
                if not first:
                    nc.vector.tensor_tensor(kv_v[:, 3:5, :t], pr[:, 3:5, :t],
                                            mm[:, 3:5, :t], op=OP.mult)

                # ---- lane 1 conv + dsu0 interleave ----
                conv_act(idx, 1, 0, conv_mm(idx, 1, 0, wsb))
                p1 = pu.tile([R, KV], f32, tag="u0", name="p1")
                nc.tensor.matmul(p1[:, :2], w1sb[:, 0, :], seq0[:, 0:2],
                                 start=True, stop=True)
                p2 = pu.tile([R, KV], f32, tag="u1", name="p2")
                nc.tensor.matmul(p2[:, :2], w1sb[:, 1, :], ht0[:, 0:2],
                                 start=True, stop=True)
                u1i = wk.tile([R, BLP], f32r, tag="u1i0", name="u1i")
                nc.scalar.activation(u1i[:, :2], p1[:, :2], AF.Relu,
                                     bias=b1sb[:, 0:1], scale=inv_hw)
                u1h = wk.tile([R, BLP], f32r, tag="u1h0", name="u1h")
                nc.scalar.activation(u1h[:, :2], p2[:, :2], AF.Relu,
                                     bias=b1sb[:, 1:2], scale=1.0)
                ps1b = conv_mm(idx, 1, 1, wsb)
                pgt = []
                for g in range(3):
                    pge = pg.tile([C, KV], f32, tag="g", name="pge")
                    nc.tensor.matmul(pge[:, :2], w2sb[:, 0, g, :], u1i[:, :2],
                                     start=True, stop=False)
                    nc.tensor.matmul(pge[:, :2], w2sb[:, 1, g, :], u1h[:, :2],
                                     start=False, stop=True)
                    pgt.append(pge)
                conv_act(idx, 1, 1, ps1b)
                seq_add(1)
                si, sf, s2 = gates_post(pgt, 1, b2tsb[:, 0:1], b2tsb[:, 1:2],
                                        b2tsb[:, 2:3], "x")
                ncx = gate_combine(si, sf, s2, ct0[:, 0:1], 1, "x")
                nc.vector.tensor_copy(ct0[:, 0:1], ncx[:, :1])
                nc.scalar.activation(ht0[:, 0:1], ct0[:, 0:1], AF.Sigmoid,
                                     bias=0.0, scale=1.0)
                cx = ct0[:, 0:1]

                # ---- lane 2 conv + cell_hh/kv-head interleave ----
                conv_act(idx, 2, 0, conv_mm(idx, 2, 0, wsb))
                pm1 = u1kv = None
                if not first:
                    pm1 = pu.tile([R, KV], f32, tag="u0", name="pm1")
                    nc.tensor.matmul(pm1[:, :nkv], w1sb[:, 0, :],
                                     kvin[:, :nkv], start=True, stop=True)
                    u1kv = wk.tile([R, KV], f32r, tag="u1kv", name="u1kv")
                    nc.scalar.activation(u1kv[:, :nkv], pm1[:, :nkv], AF.Relu,
                                         bias=b1sb[:, 0:1], scale=inv_hw)
                ps2b = conv_mm(idx, 2, 1, wsb)
                if not first:
                    ph1 = pu.tile([R, KV], f32, tag="u1", name="ph1")
                    nc.tensor.matmul(ph1[:, :2], w1sb[:, 1, :], ht0[:, 0:2],
                                     start=True, stop=True)
                    u1hx = wk.tile([R, 2], f32r, tag="u1hx", name="u1hx")
                    nc.scalar.activation(u1hx[:], ph1[:, :2], AF.Relu,
                                         bias=b1sb[:, 1:2], scale=1.0)
                conv_act(idx, 2, 1, ps2b)
                seq_add(2)

                # ---- lane 3 conv + kv gates interleave ----
                ps3a = conv_mm(idx, 3, 0, wsb)
                if not first:
                    biask = wk.tile([C, 3], f32, tag="biask", name="biask")
                    for g in range(3):
                        phg = pg.tile([C, KV], f32, tag="g", name="phg")
                        nc.tensor.matmul(phg[:, :2], w2sb[:, 1, g, :],
                                         u1hx[:], start=True, stop=True)
                        if g == 2:
                            nc.vector.scalar_tensor_tensor(
                                biask[:, 2:3], phg[:, 0:1], 2.0,
                                b2tsb[:, 2:3], op0=OP.mult, op1=OP.add)
                        else:
                            nc.vector.tensor_tensor(
                                biask[:, g:g + 1], phg[:, 0:1],
                                b2tsb[:, g:g + 1], op=OP.add)
                    pgt = []
                    for g in range(3):
                        pge = pg.tile([C, KV], f32, tag="g", name="pge")
                        nc.tensor.matmul(pge[:, :nkv], w2sb[:, 0, g, :],
                                         u1kv[:, :nkv], start=True, stop=True)
                        pgt.append(pge)
                conv_act(idx, 3, 0, ps3a)
                if not first:
                    si, sf, s2 = gates_post(pgt, nkv, biask[:, 0:1],
                                            biask[:, 1:2], biask[:, 2:3],
                                            "kv")
                    ncx = gate_combine(si, sf, s2, cx, nkv, "kv")
                    khvh = wk.tile([C, KV], f32, tag="khvh", name="khvh")
                    nc.scalar.activation(khvh[:, :nkv], ncx[:, :nkv],
                                         AF.Sigmoid, bias=0.0, scale=1.0)
                    kh = khvh[:, :nk].rearrange("p (b t) -> p b t", b=BL)
                    vh = khvh[:, nk:nkv].rearrange("p (b t) -> p b t", b=BL)
                    nc.vector.tensor_tensor(kr[:, :, :t], kr[:, :, :t],
                                            kh[:, :, :t], op=OP.mult)
                    nc.vector.tensor_tensor(mm[:, :, :t], mm[:, :, :t],
                                            vh[:, :, :t], op=OP.mult)
                conv_act(idx, 3, 1, conv_mm(idx, 3, 1, wsb))
                seq_add(3)

                if not first:
                    # lanes 0-2 softmax + lane1 context (DVE)
                    softmax_group(idx, 0, 3)
                    ctx_dve_chain(idx, 1, not last)
                    if last:
                        nc.sync.dma_start(OUT[:, 1], xp[n_layers][:, 1])

                # ---- lane 4 conv ----
                conv_act(idx, 4, 0, conv_mm(idx, 4, 0, wsb))
                dgs0 = None
                if not first and not last:
                    dgs0 = ctx_diags(idx, 0)   # Pool builds lane-0 diags
                conv_act(idx, 4, 1, conv_mm(idx, 4, 1, wsb))
                seq_add(4)

                if not first and last:
                    softmax_group(idx, 3, 5)

                if not first:
                    ctx_dve_chain(idx, 2, not last)
                    if last:
                        nc.sync.dma_start(OUT[:, 2], xp[n_layers][:, 2])
                    if not last:
                        ctx_pe_mm(idx, 0, dgs0, True)   # lane-0 ctx on PE

                if not first and not last:
                    softmax_group(idx, 3, 5)

                if not last:
                    dsu_update(seq, 1, 5, 4)
                    # append this layer's q / modulation slot
                    nc.vector.tensor_copy(kr[:, 0, idx:idx + 1], seq0[:, 0:1])
                    nc.vector.tensor_copy(kr[:, 1:5, idx], seq[:, 1:5])
                    nc.gpsimd.memset(mm[:, :, idx], 1.0)
                    if first:
                        nc.vector.tensor_copy(pr[:, 0, idx:idx + 1],
                                              seq0[:, 0:1])
                        nc.vector.tensor_copy(pr[:, 1:5, idx], seq[:, 1:5])

                if not first:
                    if last:
                        # drain: lanes 3,4 on PE (diags Act / Pool in parallel)
                        dgs3 = ctx_diags(idx, 3, engines="A")
                        dgs4 = ctx_diags(idx, 4, engines="P")
                        ctx_pe_mm(idx, 3, dgs3, False)
                        nc.sync.dma_start(OUT[:, 3], xp[n_layers][:, 3])
                        ctx_pe_mm(idx, 4, dgs4, False)
                        nc.sync.dma_start(OUT[:, 4], xp[n_layers][:, 4])
                    else:
                        def mk(bb):
                            def emit():
                                ctx_dve_chain(idx, bb, True)
                            return emit
                        deferred = [mk(3), mk(4)]
                return wsb_next, deferred

            wsb = wsb0
            deferred = []
            for idx in range(n_layers):
                wsb, deferred = emit_layer(idx, wsb, deferred)
            if n_layers == 1:
                for b in range(1, BL):
                    nc.sync.dma_start(OUT[:, b], xp[1][:, b])

    nc.compile()
    return nc


def prep_inputs(x, conv_w, conv_b, ih_w1, ih_b1, ih_w2, ih_b2,
                hh_w1, hh_b1, hh_w2, hh_b2):
    """Host-side prep: pad/transpose into the kernel's layouts."""
    f = np.float32
    if KF32:
        xf = f
    else:
        import ml_dtypes
        xf = np.dtype(ml_dtypes.bfloat16)
    xt = np.ascontiguousarray(np.transpose(np.asarray(x, f), (1, 0, 2, 3)))
    cw = np.ascontiguousarray(
        np.transpose(np.asarray(conv_w, f), (2, 0, 3, 4, 1))
        .reshape(C, L, 9, C).astype(xf))
    cb = np.ascontiguousarray(np.asarray(conv_b, f).T)             # [C, L]
    w1 = np.ascontiguousarray(
        np.stack([np.asarray(ih_w1, f).T, np.asarray(hh_w1, f).T], axis=1))
    b1 = np.ascontiguousarray(
        np.stack([np.asarray(ih_b1, f), np.asarray(hh_b1, f)], axis=1))
    w2 = np.ascontiguousarray(
        np.stack([np.asarray(ih_w2, f).T.reshape(R, 3, C),
                  np.asarray(hh_w2, f).T.reshape(R, 3, C)], axis=1))
    b2t = (np.asarray(ih_b2, f) + np.asarray(hh_b2, f)).reshape(3, C).T.copy()
    b2t[:, 2] *= 2.0   # tanh-as-sigmoid trick needs the c bias pre-doubled
    ones = np.ones((C, C), f)
    eye = np.ascontiguousarray(np.eye(C, dtype=f).astype(xf))
    shards = []
    for k in range(NCORES):
        lanes = [0, 4 * k, 4 * k + 1, 4 * k + 2, 4 * k + 3]
        xs = np.zeros((C, BL, HP, WP), xf)
        xs[:, :, 1:1 + H, WL:WL + W] = xt[:, lanes].astype(xf)
        shards.append(np.ascontiguousarray(xs))
    common = {"cw": cw, "cb": cb, "w1": w1, "b1": b1, "w2": w2,
              "b2t": np.ascontiguousarray(b2t), "ones": ones, "eye": eye}
    return [dict(common, x0=shards[k]) for k in range(NCORES)]


def gather_out(results):
    """results: list of per-core dicts with 'out' [C, BL, HP, WP]."""
    out = np.empty((B, C, H, W), np.float32)
    for k in range(NCORES):
        o = np.asarray(results[k]["out"][:, 1:5, 1:1 + H, WL:WL + W],
                       np.float32)
        out[4 * k:4 * k + 4] = np.transpose(o, (1, 0, 2, 3))
    return out


_NC_CACHE = {}


def kernel(**inputs) -> np.ndarray:
    from concourse.bass_utils import run_bass_kernel_spmd

    if "nc" not in _NC_CACHE:
        _NC_CACHE["nc"] = build_nc()
    nc = _NC_CACHE["nc"]
    in_maps = prep_inputs(**inputs)
    res = run_bass_kernel_spmd(nc, in_maps, core_ids=list(range(NCORES)))
    return gather_out(res.results)


if __name__ == "__main__":
    nc = build_nc()
    print("built ok")


# revision 7
# speedup vs baseline: 1.1962x; 1.1962x over previous
"""Trainium2 Bass kernel for nn_Attention_40346922778795 (v2 schedule).

8 layers of: conv3x3+ReLU -> GAP -> DSU recurrence; k/v modulation driven by
batch-0 hidden state; additive attention over layer history.

Distribution: batch data-parallel, 4 owned batches + 1 replicated batch-0
lane per core (hidden[0][0] dependency stays local; zero collectives).

v2 schedule changes vs v1 baseline (301us):
 - conv activations write straight into the next xp buffer; the attention
   context accumulates IN-PLACE there (no xc staging tile).
 - bf16 feature maps by default (KF32=1 env falls back to fp32r): DVE AXPYs
   run in 2x packed mode, LDWEIGHTS halves, DMA halves.
 - softmax exp computed as sigmoid ratio e^s = p/(1-p), keeping ScalarE in
   the sigmoid activation table all kernel long (no ACT_TABLE_LOAD thrash).
 - DSU / cell_hh / kv-gate matmul chains interleaved INTO the conv matmul
   stream so TensorE never idles waiting on them and stays at high p-state.
 - per-lane context routing: lane0 on TensorE (diag matmuls, feeds next
   layer's first conv ASAP), lanes 1-2 DVE inline, lanes 3-4 DVE deferred
   into the next layer's stream (their convs run late, so there is slack).
 - last layer: lane-0 context + store skipped (replica lane is dead there),
   lane4 context on TensorE to shorten the DVE drain.
"""

import os
import sys

for _p in ("/opt/trn_rl_repo",):
    if _p not in sys.path and os.path.isdir(_p):
        sys.path.insert(0, _p)
os.environ.setdefault("MYCRO_LOCAL_CACHE", "1")

import numpy as np  # noqa: E402

B, C, H, W, L = 32, 128, 28, 28, 8
R = C // 4
BL = 5          # lanes per core: [batch0 replica, 4 owned batches]
KF32 = bool(os.environ.get("KF32"))
HP = H + 2
# bf16 x-buffers use left-pad 2 so every 28-wide interior row starts on a
# 4-byte boundary (DVE 2x packed mode requirement); fp32r keeps pad 1.
WL = 1 if KF32 else 2
WP = 30 if KF32 else 32
NCORES = 8
CH = 14         # conv chunk rows
BLP = 6         # lane count padded to even (fp32r matmul ISA restriction)
NCHUNK = H // CH


def build_nc(n_layers=L):
    import concourse.bacc as bacc
    import concourse.mybir as mybir
    import concourse.tile as tile

    dt = mybir.dt
    AF = mybir.ActivationFunctionType
    OP = mybir.AluOpType
    f32, f32r = dt.float32, dt.float32r
    xdt = f32r if KF32 else dt.bfloat16

    nc = bacc.Bacc()

    X0 = nc.dram_tensor("x0", [C, BL, HP, WP], xdt, kind="ExternalInput")
    CW = nc.dram_tensor("cw", [C, L, 9, C], xdt, kind="ExternalInput")
    CB = nc.dram_tensor("cb", [C, L], f32, kind="ExternalInput")
    W1 = nc.dram_tensor("w1", [C, 2, R], f32r, kind="ExternalInput")
    B1 = nc.dram_tensor("b1", [R, 2], f32, kind="ExternalInput")
    W2 = nc.dram_tensor("w2", [R, 2, 3, C], f32r, kind="ExternalInput")
    # b2t columns: [i, f, 2*c] (c-gate bias pre-doubled for the tanh trick)
    B2T = nc.dram_tensor("b2t", [C, 3], f32, kind="ExternalInput")
    ONES = nc.dram_tensor("ones", [C, C], f32r, kind="ExternalInput")
    EYE = nc.dram_tensor("eye", [C, C], xdt, kind="ExternalInput")
    OUT = nc.dram_tensor("out", [C, BL, HP, WP], xdt, kind="ExternalOutput")

    inv_hw = 1.0 / (H * W)
    exp_scale = 1.0 / (float(H * W) ** 2 * np.sqrt(np.float32(C)))
    KV = 2 * L * BL   # width of the merged k/v modulation block

    with tile.TileContext(nc) as tc:
        with tc.tile_pool(name="xpad", bufs=1) as xpool, \
             tc.tile_pool(name="wts", bufs=2) as wpool, \
             tc.tile_pool(name="small", bufs=1) as spool, \
             tc.tile_pool(name="work", bufs=2) as wk, \
             tc.tile_pool(name="dpool", bufs=8) as dpool, \
             tc.tile_pool(name="pc", bufs=3, space="PSUM") as pc, \
             tc.tile_pool(name="pg", bufs=3, space="PSUM") as pg, \
             tc.tile_pool(name="pu", bufs=1, space="PSUM") as pu:

            # ---- persistent state (attention tensors are [C, BL, L]) ----
            xp = [xpool.tile([C, BL, HP, WP], xdt, tag=f"xp{i}", name=f"xp{i}")
                  for i in range(L)]
            xp.append(xp[0])  # layer-8 output reuses the input buffer
            kr = spool.tile([C, BL, L], f32r, tag="kr")     # raw q sums, modulated
            mm = spool.tile([C, BL, L], f32, tag="mm")      # running v modulation
            pr = spool.tile([C, BL, L], f32, tag="pr")      # raw pooled-v sums
            w1sb = spool.tile([C, 2, R], f32r, tag="w1sb")
            b1sb = spool.tile([R, 2], f32, tag="b1sb")
            w2sb = spool.tile([R, 2, 3, C], f32r, tag="w2sb")
            b2tsb = spool.tile([C, 3], f32, tag="b2t")
            cbsb = spool.tile([C, L], f32, tag="cbsb")
            ones = spool.tile([C, C], f32r, tag="ones")
            eye = spool.tile([C, C], xdt, tag="eye")
            ht = spool.tile([C, BLP], f32r, tag="ht")
            ct = spool.tile([C, BL], f32, tag="ct")
            ht0 = spool.tile([C, 2], f32r, tag="ht0")
            ct0 = spool.tile([C, 1], f32, tag="ct0")
            seq = spool.tile([C, BLP], f32r, tag="seq")
            seq0 = spool.tile([C, 2], f32r, tag="seq0")
            kvin = spool.tile([C, KV], f32r, tag="kvin")
            qk = spool.tile([C, BL, L], f32r, tag="qk")
            sig = spool.tile([C, BL, L], f32, tag="sig")
            om = spool.tile([C, BL, L], f32, tag="om")
            rr = spool.tile([C, BL, L], f32, tag="rr")
            ee = spool.tile([C, BL, L], f32, tag="ee")
            den = spool.tile([C, BL], f32, tag="den")
            dinv = spool.tile([C, BL], f32, tag="dinv")
            aw = spool.tile([C, BL, L], f32, tag="aw")
            gsum = spool.tile([C, BL, NCHUNK], f32, tag="gsum")
            pacc = spool.tile([C, BL, NCHUNK], f32, tag="pacc")

            # ---- one-time loads & init ----
            wsb0 = wpool.tile([C, 9, C], xdt, tag="wsb", name="wsb0")
            nc.sync.dma_start(wsb0[:], CW[:, 0])
            for b in range(BL):
                nc.sync.dma_start(xp[0][:, b], X0[:, b])
            nc.sync.dma_start(w1sb[:], W1[:])
            nc.sync.dma_start(b1sb[:], B1[:])
            nc.sync.dma_start(w2sb[:], W2[:])
            nc.sync.dma_start(b2tsb[:], B2T[:])
            nc.sync.dma_start(cbsb[:], CB[:])
            nc.sync.dma_start(ones[:], ONES[:])
            nc.sync.dma_start(eye[:], EYE[:])
            nc.gpsimd.memset(ht[:].bitcast(f32), 0.0)
            nc.gpsimd.memset(ct[:], 0.0)
            nc.gpsimd.memset(ht0[:].bitcast(f32), 0.0)
            nc.gpsimd.memset(ct0[:], 0.0)
            nc.gpsimd.memset(seq[:].bitcast(f32), 0.0)
            nc.gpsimd.memset(seq0[:].bitcast(f32), 0.0)
            nc.gpsimd.memset(kvin[:].bitcast(f32), 0.0)
            nc.gpsimd.memset(kr[:].bitcast(f32), 0.0)
            nc.gpsimd.memset(qk[:].bitcast(f32), 0.0)
            nc.gpsimd.memset(mm[:], 0.0)
            nc.gpsimd.memset(pr[:], 0.0)
            # pad borders of buffers 1..7 are zeroed lazily, one buffer per
            # layer, inside emit_layer (keeps Pool off the startup critical
            # path; buffer 8 aliases the host-zero-padded input buffer)
            bdt = f32 if KF32 else dt.uint16

            def zero_borders(i):
                nc.gpsimd.memset(xp[i][:, :, 0, :].bitcast(bdt), 0)
                nc.gpsimd.memset(xp[i][:, :, HP - 1, :].bitcast(bdt), 0)
                nc.gpsimd.memset(xp[i][:, :, 1:HP - 1, 0:WL].bitcast(bdt), 0)
                nc.gpsimd.memset(xp[i][:, :, 1:HP - 1, WL + W:WP].bitcast(bdt), 0)
            zero_borders(1)

            def conv_mm(idx, b, ch, wsb):
                h0 = ch * CH
                ps = pc.tile([C, CH, W], f32, tag="c", name="ps")
                for tap in range(9):
                    dy, dx = tap // 3, tap % 3
                    rhs = xp[idx][:, b, h0 + dy:h0 + dy + CH,
                                  dx + WL - 1:dx + WL - 1 + W]
                    nc.tensor.matmul(ps[:], wsb[:, tap, :], rhs,
                                     start=(tap == 0), stop=(tap == 8))
                return ps

            def conv_act(idx, b, ch, ps):
                h0 = ch * CH
                dst = xp[idx + 1][:, b, 1 + h0:1 + h0 + CH, WL:WL + W]
                nc.scalar.activation(dst, ps[:], AF.Relu,
                                     bias=cbsb[:, idx:idx + 1], scale=1.0,
                                     accum_out=gsum[:, b, ch:ch + 1])

            def seq_add(b):
                dst = seq0[:, 0:1] if b == 0 else seq[:, b:b + 1]
                with nc.allow_low_precision("f32r rounding of f32 sum"):
                    nc.vector.tensor_tensor(dst, gsum[:, b, 0:1].bitcast(f32r),
                                            gsum[:, b, 1:2].bitcast(f32r),
                                            op=OP.add)

            def gates_post(pgt, nw, bias_i, bias_f, bias_c2, name):
                """sigmoid(i), sigmoid(f), sigmoid(2c+2bc) on [C, nw]."""
                si = wk.tile([C, max(nw, BL)], f32, tag=f"si{name}", name="si")
                sf = wk.tile([C, max(nw, BL)], f32, tag=f"sf{name}", name="sf")
                s2 = wk.tile([C, max(nw, BL)], f32, tag=f"s2{name}", name="s2")
                nc.scalar.activation(si[:, :nw], pgt[0][:, :nw], AF.Sigmoid,
                                     bias=bias_i)
                nc.scalar.activation(sf[:, :nw], pgt[1][:, :nw], AF.Sigmoid,
                                     bias=bias_f)
                nc.scalar.activation(s2[:, :nw], pgt[2][:, :nw], AF.Sigmoid,
                                     bias=bias_c2, scale=2.0)
                return si, sf, s2

            def gate_combine(si, sf, s2, cx_ap, nw, name):
                """ncx = sf*cx + si*(2*s2-1); returns [C, nw] f32 tile."""
                tmp = wk.tile([C, max(nw, BL)], f32, tag=f"tm{name}", name="tmp")
                nc.vector.scalar_tensor_tensor(tmp[:, :nw], s2[:, :nw], 2.0,
                                               si[:, :nw], op0=OP.mult,
                                               op1=OP.mult)
                ncx = wk.tile([C, max(nw, BL)], f32, tag=f"nx{name}", name="ncx")
                if cx_ap.free_size() == 1:
                    nc.vector.scalar_tensor_tensor(ncx[:, :nw], sf[:, :nw],
                                                   cx_ap, tmp[:, :nw],
                                                   op0=OP.mult, op1=OP.add)
                else:
                    t2 = wk.tile([C, max(nw, BL)], f32, tag=f"t2{name}",
                                 name="t2")
                    nc.vector.tensor_tensor(t2[:, :nw], sf[:, :nw], cx_ap,
                                            op=OP.mult)
                    nc.vector.tensor_tensor(ncx[:, :nw], t2[:, :nw], tmp[:, :nw],
                                            op=OP.add)
                nc.vector.tensor_tensor(ncx[:, :nw], ncx[:, :nw], si[:, :nw],
                                        op=OP.subtract)
                return ncx

            def dsu_update(sq, lo, hi, nw):
                """DSU over input columns [lo:hi) (hi-lo even); writes the
                first nw lanes' ht/ct columns [lo:lo+nw)."""
                w_ = hi - lo
                p1 = pu.tile([R, KV], f32, tag="u0", name="p1")
                nc.tensor.matmul(p1[:, :w_], w1sb[:, 0, :], sq[:, lo:hi],
                                 start=True, stop=True)
                p2 = pu.tile([R, KV], f32, tag="u1", name="p2")
                nc.tensor.matmul(p2[:, :w_], w1sb[:, 1, :], ht[:, lo:hi],
                                 start=True, stop=True)
                u1i = wk.tile([R, BLP], f32r, tag="u1i", name="u1i")
                nc.scalar.activation(u1i[:, :w_], p1[:, :w_], AF.Relu,
                                     bias=b1sb[:, 0:1], scale=inv_hw)
                u1h = wk.tile([R, BLP], f32r, tag="u1h", name="u1h")
                nc.scalar.activation(u1h[:, :w_], p2[:, :w_], AF.Relu,
                                     bias=b1sb[:, 1:2], scale=1.0)
                pgt = []
                for g in range(3):
                    pge = pg.tile([C, KV], f32, tag="g", name="pge")
                    nc.tensor.matmul(pge[:, :w_], w2sb[:, 0, g, :], u1i[:, :w_],
                                     start=True, stop=False)
                    nc.tensor.matmul(pge[:, :w_], w2sb[:, 1, g, :], u1h[:, :w_],
                                     start=False, stop=True)
                    pgt.append(pge)
                si, sf, s2 = gates_post(pgt, nw, b2tsb[:, 0:1],
                                        b2tsb[:, 1:2], b2tsb[:, 2:3], "x")
                ncx = gate_combine(si, sf, s2, ct[:, lo:lo + nw], nw, "x")
                nc.vector.tensor_copy(ct[:, lo:lo + nw], ncx[:, :nw])
                nc.scalar.activation(ht[:, lo:lo + nw], ct[:, lo:lo + nw],
                                     AF.Sigmoid, bias=0.0, scale=1.0)

            def softmax_group(idx, s0, s1):
                """qk -> scores -> sigmoid-ratio softmax -> aw for lanes
                [s0, s1)."""
                t, tp = idx, idx + (idx % 2)
                w_ = s1 - s0
                for b in range(s0, s1):
                    sqcol = seq0[:, 0:1] if b == 0 else seq[:, b:b + 1]
                    nc.vector.tensor_tensor(qk[:, b, :t], kr[:, b, :t],
                                            sqcol.broadcast_to([C, t]),
                                            op=OP.mult)
                psc = pg.tile([C, KV], f32, tag="g", name="psc")
                p3 = psc[:, :w_ * tp].rearrange("p (b t) -> p b t", b=w_)
                nc.tensor.matmul(p3[:], ones[:], qk[:, s0:s1, 0:tp],
                                 start=True, stop=True)
                nc.scalar.activation(sig[:, s0:s1, :t], p3[:, :, :t],
                                     AF.Sigmoid, bias=0.0, scale=exp_scale)
                one3 = ones[:, 0:w_ * t].bitcast(f32).rearrange(
                    "p (b t) -> p b t", b=w_)
                nc.vector.scalar_tensor_tensor(om[:, s0:s1, :t],
                                               sig[:, s0:s1, :t], -1.0,
                                               one3, op0=OP.mult, op1=OP.add)
                nc.vector.reciprocal(rr[:, s0:s1, :t], om[:, s0:s1, :t])
                nc.vector.tensor_tensor(ee[:, s0:s1, :t], sig[:, s0:s1, :t],
                                        rr[:, s0:s1, :t], op=OP.mult)
                nc.vector.tensor_reduce(den[:, s0:s1], ee[:, s0:s1, :t],
                                        axis=mybir.AxisListType.X, op=OP.add)
                nc.vector.reciprocal(dinv[:, s0:s1], den[:, s0:s1])
                for b in range(s0, s1):
                    nc.vector.tensor_tensor(om[:, b, :t], ee[:, b, :t],
                                            mm[:, b, :t], op=OP.mult)
                    nc.vector.tensor_tensor(
                        aw[:, b, :t], om[:, b, :t],
                        dinv[:, b:b + 1].broadcast_to([C, t]), op=OP.mult)


            def ctx_dve_chain(idx, b, accum):
                t = idx
                dst = xp[idx + 1][:, b, 1:1 + H, WL:WL + W]
                for ti in range(t):
                    src = xp[ti + 1][:, b, 1:1 + H, WL:WL + W]
                    kwargs = {}
                    if ti == t - 1 and accum:
                        kwargs["accum_out"] = pr[:, b, idx:idx + 1]
                    nc.vector.scalar_tensor_tensor(dst, src,
                                                   aw[:, b, ti:ti + 1], dst,
                                                   op0=OP.mult, op1=OP.add,
                                                   **kwargs)

            def ctx_diags(idx, b, engines="AAP"):
                t = idx
                dgs = []
                for ti in range(t):
                    dg = dpool.tile([C, C], xdt, tag="diag", name="dg")
                    if engines[ti % len(engines)] == "P":
                        nc.gpsimd.tensor_tensor(
                            dg[:], eye[:],
                            aw[:, b, ti:ti + 1].broadcast_to([C, C]),
                            op=OP.mult)
                    else:
                        nc.scalar.mul(dg[:], eye[:], aw[:, b, ti:ti + 1])
                    dgs.append(dg)
                return dgs

            def ctx_pe_mm(idx, b, dgs, accum):
                t = idx
                for ch in range(NCHUNK):
                    h0 = ch * CH
                    dst = xp[idx + 1][:, b, 1 + h0:1 + h0 + CH, WL:WL + W]
                    ps = pc.tile([C, CH, W], f32, tag="c", name="psx")
                    nc.tensor.matmul(ps[:], eye[:], dst, start=True, stop=False)
                    for k in range(t):
                        nc.tensor.matmul(ps[:], dgs[k][:],
                                         xp[k + 1][:, b, 1 + h0:1 + h0 + CH,
                                                   WL:WL + W],
                                         start=False, stop=(k == t - 1))
                    kwargs = {}
                    if accum:
                        kwargs["accum_out"] = pacc[:, b, ch:ch + 1]
                    nc.scalar.activation(dst, ps[:], AF.Copy, bias=0.0,
                                         scale=1.0, **kwargs)
                if accum:
                    nc.vector.tensor_reduce(
                        pr[:, b, idx:idx + 1],
                        pacc[:, b].rearrange("p (x c) -> p x c", x=1),
                        axis=mybir.AxisListType.X, op=OP.add)

            def emit_layer(idx, wsb, deferred):
                t = idx
                tp = t + (t % 2)
                last = idx == n_layers - 1
                first = idx == 0
                nk = BL * tp
                nkv = 2 * nk

                wsb_next = None
                if idx + 1 < n_layers:
                    wsb_next = wpool.tile([C, 9, C], xdt, tag="wsb",
                                          name="wsbn")
                    nc.sync.dma_start(wsb_next[:], CW[:, idx + 1])
                if 2 + idx < L:
                    zero_borders(idx + 2)   # borders needed by layer idx+1

                # ---- lane 0 conv ----
                conv_act(idx, 0, 0, conv_mm(idx, 0, 0, wsb))
                conv_act(idx, 0, 1, conv_mm(idx, 0, 1, wsb))
                seq_add(0)

                kv_k = kv_v = None
                if not first:
                    # kvin: [ k block | pooled*m block ], lanes 0-2 of the v
                    # block now; lanes 3-4 after the deferred chains below.
                    kv_k = kvin[:, :nk].rearrange("p (b t) -> p b t", b=BL)
                    kv_v = kvin[:, nk:nkv].rearrange("p (b t) -> p b t", b=BL)
                    nc.vector.tensor_copy(kv_k[:, :, :t], kr[:, :, :t])
                    nc.vector.tensor_tensor(kv_v[:, 0:3, :t], pr[:, 0:3, :t],
                                            mm[:, 0:3, :t], op=OP.mult)
                # deferred lane-3/4 context chains from the previous layer
                for fn in deferred:
                    fn()
                deferred = []
                if not first:
                    nc.vector.tensor_tensor(kv_v[:, 3:5, :t], pr[:, 3:5, :t],
                                            mm[:, 3:5, :t], op=OP.mult)

                # ---- lane 1 conv + dsu0 interleave ----
                conv_act(idx, 1, 0, conv_mm(idx, 1, 0, wsb))
                p1 = pu.tile([R, KV], f32, tag="u0", name="p1")
                nc.tensor.matmul(p1[:, :2], w1sb[:, 0, :], seq0[:, 0:2],
                                 start=True, stop=True)
                p2 = pu.tile([R, KV], f32, tag="u1", name="p2")
                nc.tensor.matmul(p2[:, :2], w1sb[:, 1, :], ht0[:, 0:2],
                                 start=True, stop=True)
                u1i = wk.tile([R, BLP], f32r, tag="u1i0", name="u1i")
                nc.scalar.activation(u1i[:, :2], p1[:, :2], AF.Relu,
                                     bias=b1sb[:, 0:1], scale=inv_hw)
                u1h = wk.tile([R, BLP], f32r, tag="u1h0", name="u1h")
                nc.scalar.activation(u1h[:, :2], p2[:, :2], AF.Relu,
                                     bias=b1sb[:, 1:2], scale=1.0)
                ps1b = conv_mm(idx, 1, 1, wsb)
                pgt = []
                for g in range(3):
                    pge = pg.tile([C, KV], f32, tag="g", name="pge")
                    nc.tensor.matmul(pge[:, :2], w2sb[:, 0, g, :], u1i[:, :2],
                                     start=True, stop=False)
                    nc.tensor.matmul(pge[:, :2], w2sb[:, 1, g, :], u1h[:, :2],
                                     start=False, stop=True)
                    pgt.append(pge)
                conv_act(idx, 1, 1, ps1b)
                seq_add(1)
                si, sf, s2 = gates_post(pgt, 1, b2tsb[:, 0:1], b2tsb[:, 1:2],
                                        b2tsb[:, 2:3], "x")
                ncx = gate_combine(si, sf, s2, ct0[:, 0:1], 1, "x")
                nc.vector.tensor_copy(ct0[:, 0:1], ncx[:, :1])
                nc.scalar.activation(ht0[:, 0:1], ct0[:, 0:1], AF.Sigmoid,
                                     bias=0.0, scale=1.0)
                cx = ct0[:, 0:1]

                # ---- lane 2 conv + cell_hh/kv-head interleave ----
                conv_act(idx, 2, 0, conv_mm(idx, 2, 0, wsb))
                pm1 = u1kv = None
                if not first:
                    pm1 = pu.tile([R, KV], f32, tag="u0", name="pm1")
                    nc.tensor.matmul(pm1[:, :nkv], w1sb[:, 0, :],
                                     kvin[:, :nkv], start=True, stop=True)
                    u1kv = wk.tile([R, KV], f32r, tag="u1kv", name="u1kv")
                    nc.scalar.activation(u1kv[:, :nkv], pm1[:, :nkv], AF.Relu,
                                         bias=b1sb[:, 0:1], scale=inv_hw)
                ps2b = conv_mm(idx, 2, 1, wsb)
                if not first:
                    ph1 = pu.tile([R, KV], f32, tag="u1", name="ph1")
                    nc.tensor.matmul(ph1[:, :2], w1sb[:, 1, :], ht0[:, 0:2],
                                     start=True, stop=True)
                    u1hx = wk.tile([R, 2], f32r, tag="u1hx", name="u1hx")
                    nc.scalar.activation(u1hx[:], ph1[:, :2], AF.Relu,
                                         bias=b1sb[:, 1:2], scale=1.0)
                conv_act(idx, 2, 1, ps2b)
                seq_add(2)

                # ---- lane 3 conv + kv gates interleave ----
                ps3a = conv_mm(idx, 3, 0, wsb)
                if not first:
                    biask = wk.tile([C, 3], f32, tag="biask", name="biask")
                    for g in range(3):
                        phg = pg.tile([C, KV], f32, tag="g", name="phg")
                        nc.tensor.matmul(phg[:, :2], w2sb[:, 1, g, :],
                                         u1hx[:], start=True, stop=True)
                        if g == 2:
                            nc.vector.scalar_tensor_tensor(
                                biask[:, 2:3], phg[:, 0:1], 2.0,
                                b2tsb[:, 2:3], op0=OP.mult, op1=OP.add)
                        else:
                            nc.vector.tensor_tensor(
                                biask[:, g:g + 1], phg[:, 0:1],
                                b2tsb[:, g:g + 1], op=OP.add)
                    pgt = []
                    for g in range(3):
                        pge = pg.tile([C, KV], f32, tag="g", name="pge")
                        nc.tensor.matmul(pge[:, :nkv], w2sb[:, 0, g, :],
                                         u1kv[:, :nkv], start=True, stop=True)
                        pgt.append(pge)
                conv_act(idx, 3, 0, ps3a)
                if not first:
                    si, sf, s2 = gates_post(pgt, nkv, biask[:, 0:1],
                                            biask[:, 1:2], biask[:, 2:3],
                                            "kv")
                    ncx = gate_combine(si, sf, s2, cx, nkv, "kv")
                    khvh = wk.tile([C, KV], f32, tag="khvh", name="khvh")
                    nc.scalar.activation(khvh[:, :nkv], ncx[:, :nkv],
                                         AF.Sigmoid, bias=0.0, scale=1.0)
                    kh = khvh[:, :nk].rearrange("p (b t) -> p b t", b=BL)
                    vh = khvh[:, nk:nkv].rearrange("p (b t) -> p b t", b=BL)
                    nc.vector.tensor_tensor(kr[:, :, :t], kr[:, :, :t],
                                            kh[:, :, :t], op=OP.mult)
                    nc.vector.tensor_tensor(mm[:, :, :t], mm[:, :, :t],
                                            vh[:, :, :t], op=OP.mult)
                conv_act(idx, 3, 1, conv_mm(idx, 3, 1, wsb))
                seq_add(3)

                if not first:
                    # lanes 0-2 softmax + lane1 context (DVE)
                    softmax_group(idx, 0, 3)
                    ctx_dve_chain(idx, 1, not last)
                    if last:
                        nc.sync.dma_start(OUT[:, 1], xp[n_layers][:, 1])

                # ---- lane 4 conv ----
                conv_act(idx, 4, 0, conv_mm(idx, 4, 0, wsb))
                dgs0 = None
                if not first and not last:
                    dgs0 = ctx_diags(idx, 0)   # Pool builds lane-0 diags
                conv_act(idx, 4, 1, conv_mm(idx, 4, 1, wsb))
                seq_add(4)

                if not first and last:
                    softmax_group(idx, 3, 5)

                if not first:
                    ctx_dve_chain(idx, 2, not last)
                    if last:
                        nc.sync.dma_start(OUT[:, 2], xp[n_layers][:, 2])
                    if not last:
                        ctx_pe_mm(idx, 0, dgs0, True)   # lane-0 ctx on PE

                if not first and not last:
                    softmax_group(idx, 3, 5)

                if not last:
                    dsu_update(seq, 1, 5, 4)
                    # append this layer's q / modulation slot
                    nc.vector.tensor_copy(kr[:, 0, idx:idx + 1], seq0[:, 0:1])
                    nc.vector.tensor_copy(kr[:, 1:5, idx], seq[:, 1:5])
                    nc.gpsimd.memset(mm[:, :, idx], 1.0)
                    if first:
                        nc.vector.tensor_copy(pr[:, 0, idx:idx + 1],
                                              seq0[:, 0:1])
                        nc.vector.tensor_copy(pr[:, 1:5, idx], seq[:, 1:5])

                if not first:
                    if last:
                        # drain: lanes 3,4 on PE (diags Act / Pool in parallel)
                        dgs3 = ctx_diags(idx, 3, engines="A")
                        dgs4 = ctx_diags(idx, 4, engines="P")
                        ctx_pe_mm(idx, 3, dgs3, False)
                        nc.sync.dma_start(OUT[:, 3], xp[n_layers][:, 3])
                        ctx_pe_mm(idx, 4, dgs4, False)
                        nc.sync.dma_start(OUT[:, 4], xp[n_layers][:, 4])
                    else:
                        def mk(bb):
                            def emit():
                                ctx_dve_chain(idx, bb, True)
                            return emit
                        deferred = [mk(3), mk(4)]
                return wsb_next, deferred

            wsb = wsb0
            deferred = []
            for idx in range(n_layers):
                wsb, deferred = emit_layer(idx, wsb, deferred)
            if n_layers == 1:
                for b in range(1, BL):
                    nc.sync.dma_start(OUT[:, b], xp[1][:, b])

    nc.compile()
    return nc


def prep_inputs(x, conv_w, conv_b, ih_w1, ih_b1, ih_w2, ih_b2,
                hh_w1, hh_b1, hh_w2, hh_b2):
    """Host-side prep: pad/transpose into the kernel's layouts."""
    f = np.float32
    if KF32:
        xf = f
    else:
        import ml_dtypes
        xf = np.dtype(ml_dtypes.bfloat16)
    xt = np.ascontiguousarray(np.transpose(np.asarray(x, f), (1, 0, 2, 3)))
    cw = np.ascontiguousarray(
        np.transpose(np.asarray(conv_w, f), (2, 0, 3, 4, 1))
        .reshape(C, L, 9, C).astype(xf))
    cb = np.ascontiguousarray(np.asarray(conv_b, f).T)             # [C, L]
    w1 = np.ascontiguousarray(
        np.stack([np.asarray(ih_w1, f).T, np.asarray(hh_w1, f).T], axis=1))
    b1 = np.ascontiguousarray(
        np.stack([np.asarray(ih_b1, f), np.asarray(hh_b1, f)], axis=1))
    w2 = np.ascontiguousarray(
        np.stack([np.asarray(ih_w2, f).T.reshape(R, 3, C),
                  np.asarray(hh_w2, f).T.reshape(R, 3, C)], axis=1))
    b2t = (np.asarray(ih_b2, f) + np.asarray(hh_b2, f)).reshape(3, C).T.copy()
    b2t[:, 2] *= 2.0   # tanh-as-sigmoid trick needs the c bias pre-doubled
    ones = np.ones((C, C), f)
    eye = np.ascontiguousarray(np.eye(C, dtype=f).astype(xf))
    shards = []
    for k in range(NCORES):
        lanes = [0, 4 * k, 4 * k + 1, 4 * k + 2, 4 * k + 3]
        xs = np.zeros((C, BL, HP, WP), xf)
        xs[:, :, 1:1 + H, WL:WL + W] = xt[:, lanes].astype(xf)
        shards.append(np.ascontiguousarray(xs))
    common = {"cw": cw, "cb": cb, "w1": w1, "b1": b1, "w2": w2,
              "b2t": np.ascontiguousarray(b2t), "ones": ones, "eye": eye}
    return [dict(common, x0=shards[k]) for k in range(NCORES)]


def gather_out(results):
    """results: list of per-core dicts with 'out' [C, BL, HP, WP]."""
    out = np.empty((B, C, H, W), np.float32)
    for k in range(NCORES):
        o = np.asarray(results[k]["out"][:, 1:5, 1:1 + H, WL:WL + W],
                       np.float32)
        out[4 * k:4 * k + 4] = np.transpose(o, (1, 0, 2, 3))
    return out


_NC_CACHE = {}


def kernel(**inputs) -> np.ndarray:
    from concourse.bass_utils import run_bass_kernel_spmd

    if "nc" not in _NC_CACHE:
        _NC_CACHE["nc"] = build_nc()
    nc = _NC_CACHE["nc"]
    in_maps = prep_inputs(**inputs)
    res = run_bass_kernel_spmd(nc, in_maps, core_ids=list(range(NCORES)))
    return gather_out(res.results)


if __name__ == "__main__":
    nc = build_nc()
    print("built ok")


# revision 14
# speedup vs baseline: 1.2722x; 1.0636x over previous
"""Trainium2 Bass kernel for nn_Attention_40346922778795 (v2 schedule).

8 layers of: conv3x3+ReLU -> GAP -> DSU recurrence; k/v modulation driven by
batch-0 hidden state; additive attention over layer history.

Distribution: batch data-parallel, 4 owned batches + 1 replicated batch-0
lane per core (hidden[0][0] dependency stays local; zero collectives).

v2 schedule changes vs v1 baseline (301us):
 - conv activations write straight into the next xp buffer; the attention
   context accumulates IN-PLACE there (no xc staging tile).
 - bf16 feature maps by default (KF32=1 env falls back to fp32r): DVE AXPYs
   run in 2x packed mode, LDWEIGHTS halves, DMA halves.
 - softmax exp computed as sigmoid ratio e^s = p/(1-p), keeping ScalarE in
   the sigmoid activation table all kernel long (no ACT_TABLE_LOAD thrash).
 - DSU / cell_hh / kv-gate matmul chains interleaved INTO the conv matmul
   stream so TensorE never idles waiting on them and stays at high p-state.
 - per-lane context routing: lane0 on TensorE (diag matmuls, feeds next
   layer's first conv ASAP), lanes 1-2 DVE inline, lanes 3-4 DVE deferred
   into the next layer's stream (their convs run late, so there is slack).
 - last layer: lane-0 context + store skipped (replica lane is dead there),
   lane4 context on TensorE to shorten the DVE drain.
"""

import os
import sys

for _p in ("/opt/trn_rl_repo",):
    if _p not in sys.path and os.path.isdir(_p):
        sys.path.insert(0, _p)
os.environ.setdefault("MYCRO_LOCAL_CACHE", "1")

import numpy as np  # noqa: E402

B, C, H, W, L = 32, 128, 28, 28, 8
R = C // 4
BL = 5          # lanes per core: [batch0 replica, 4 owned batches]
KF32 = bool(os.environ.get("KF32"))
HP = H + 2
# bf16 x-buffers use left-pad 2 so every 28-wide interior row starts on a
# 4-byte boundary (DVE 2x packed mode requirement); fp32r keeps pad 1.
WL = 1 if KF32 else 2
WP = 30 if KF32 else 32
NCORES = 8
CH = 14         # conv chunk rows
BLP = 6         # lane count padded to even (fp32r matmul ISA restriction)
NCHUNK = H // CH


def build_nc(n_layers=L):
    import concourse.bacc as bacc
    import concourse.mybir as mybir
    import concourse.tile as tile

    dt = mybir.dt
    AF = mybir.ActivationFunctionType
    OP = mybir.AluOpType
    f32, f32r = dt.float32, dt.float32r
    xdt = f32r if KF32 else dt.bfloat16

    nc = bacc.Bacc()

    X0 = nc.dram_tensor("x0", [C, BL, HP, WP], xdt, kind="ExternalInput")
    CW = nc.dram_tensor("cw", [C, L, 9, C], xdt, kind="ExternalInput")
    CB = nc.dram_tensor("cb", [C, L], f32, kind="ExternalInput")
    W1 = nc.dram_tensor("w1", [C, 2, R], f32r, kind="ExternalInput")
    B1 = nc.dram_tensor("b1", [R, 2], f32, kind="ExternalInput")
    W2 = nc.dram_tensor("w2", [R, 2, 3, C], f32r, kind="ExternalInput")
    # b2t columns: [i, f, 2*c] (c-gate bias pre-doubled for the tanh trick)
    B2T = nc.dram_tensor("b2t", [C, 3], f32, kind="ExternalInput")
    ONES = nc.dram_tensor("ones", [C, C], f32r, kind="ExternalInput")
    EYE = nc.dram_tensor("eye", [C, C], xdt, kind="ExternalInput")
    OUT = nc.dram_tensor("out", [C, BL, HP, WP], xdt, kind="ExternalOutput")

    inv_hw = 1.0 / (H * W)
    exp_scale = 1.0 / (float(H * W) ** 2 * np.sqrt(np.float32(C)))
    KV = 2 * L * BL   # width of the merged k/v modulation block

    with tile.TileContext(nc) as tc:
        with tc.tile_pool(name="xpad", bufs=1) as xpool, \
             tc.tile_pool(name="wts", bufs=2) as wpool, \
             tc.tile_pool(name="small", bufs=1) as spool, \
             tc.tile_pool(name="work", bufs=2) as wk, \
             tc.tile_pool(name="dpool", bufs=8) as dpool, \
             tc.tile_pool(name="pc", bufs=3, space="PSUM") as pc, \
             tc.tile_pool(name="pg", bufs=3, space="PSUM") as pg, \
             tc.tile_pool(name="pu", bufs=1, space="PSUM") as pu:

            # ---- persistent state (attention tensors are [C, BL, L]) ----
            xp = [xpool.tile([C, BL, HP, WP], xdt, tag=f"xp{i}", name=f"xp{i}")
                  for i in range(L)]
            xp.append(xp[0])  # layer-8 output reuses the input buffer
            kr = spool.tile([C, BL, L], f32r, tag="kr")     # raw q sums, modulated
            mm = spool.tile([C, BL, L], f32, tag="mm")      # running v modulation
            pr = spool.tile([C, BL, L], f32, tag="pr")      # raw pooled-v sums
            w1sb = spool.tile([C, 2, R], f32r, tag="w1sb")
            b1sb = spool.tile([R, 2], f32, tag="b1sb")
            w2sb = spool.tile([R, 2, 3, C], f32r, tag="w2sb")
            b2tsb = spool.tile([C, 3], f32, tag="b2t")
            cbsb = spool.tile([C, L], f32, tag="cbsb")
            ones = spool.tile([C, C], f32r, tag="ones")
            eye = spool.tile([C, C], xdt, tag="eye")
            ht = spool.tile([C, BLP], f32r, tag="ht")
            ct = spool.tile([C, BL], f32, tag="ct")
            ht0 = spool.tile([C, 2], f32r, tag="ht0")
            ct0 = spool.tile([C, 1], f32, tag="ct0")
            seq = spool.tile([C, BLP], f32r, tag="seq")
            seq0 = spool.tile([C, 2], f32r, tag="seq0")
            kvin = spool.tile([C, KV], f32r, tag="kvin")
            qk = spool.tile([C, BL, L], f32r, tag="qk")
            sig = spool.tile([C, BL, L], f32, tag="sig")
            om = spool.tile([C, BL, L], f32, tag="om")
            rr = spool.tile([C, BL, L], f32, tag="rr")
            ee = spool.tile([C, BL, L], f32, tag="ee")
            den = spool.tile([C, BL], f32, tag="den")
            dinv = spool.tile([C, BL], f32, tag="dinv")
            aw = spool.tile([C, BL, L], f32, tag="aw")
            gsum = spool.tile([C, BL, NCHUNK], f32, tag="gsum")
            pacc = spool.tile([C, BL, NCHUNK], f32, tag="pacc")

            # ---- one-time loads & init ----
            wsb0 = wpool.tile([C, 9, C], xdt, tag="wsb", name="wsb0")
            nc.sync.dma_start(wsb0[:], CW[:, 0])
            for b in range(BL):
                nc.sync.dma_start(xp[0][:, b], X0[:, b])
            nc.sync.dma_start(w1sb[:], W1[:])
            nc.sync.dma_start(b1sb[:], B1[:])
            nc.sync.dma_start(w2sb[:], W2[:])
            nc.sync.dma_start(b2tsb[:], B2T[:])
            nc.sync.dma_start(cbsb[:], CB[:])
            nc.sync.dma_start(ones[:], ONES[:])
            nc.sync.dma_start(eye[:], EYE[:])
            nc.gpsimd.memset(ht[:].bitcast(f32), 0.0)
            nc.gpsimd.memset(ct[:], 0.0)
            nc.gpsimd.memset(ht0[:].bitcast(f32), 0.0)
            nc.gpsimd.memset(ct0[:], 0.0)
            nc.gpsimd.memset(seq[:].bitcast(f32), 0.0)
            nc.gpsimd.memset(seq0[:].bitcast(f32), 0.0)
            nc.gpsimd.memset(kvin[:].bitcast(f32), 0.0)
            nc.gpsimd.memset(kr[:].bitcast(f32), 0.0)
            nc.gpsimd.memset(qk[:].bitcast(f32), 0.0)
            nc.gpsimd.memset(mm[:], 0.0)
            nc.gpsimd.memset(pr[:], 0.0)
            # pad borders of buffers 1..7 are zeroed lazily, one buffer per
            # layer, inside emit_layer (keeps Pool off the startup critical
            # path; buffer 8 aliases the host-zero-padded input buffer)
            bdt = f32 if KF32 else dt.uint16

            def zero_borders(i):
                nc.gpsimd.memset(xp[i][:, :, 0, :].bitcast(bdt), 0)
                nc.gpsimd.memset(xp[i][:, :, HP - 1, :].bitcast(bdt), 0)
                nc.gpsimd.memset(xp[i][:, :, 1:HP - 1, 0:WL].bitcast(bdt), 0)
                nc.gpsimd.memset(xp[i][:, :, 1:HP - 1, WL + W:WP].bitcast(bdt), 0)
            zero_borders(1)

            def conv_mm(idx, b, ch, wsb):
                h0 = ch * CH
                ps = pc.tile([C, CH, W], f32, tag="c", name="ps")
                for tap in range(9):
                    dy, dx = tap // 3, tap % 3
                    rhs = xp[idx][:, b, h0 + dy:h0 + dy + CH,
                                  dx + WL - 1:dx + WL - 1 + W]
                    nc.tensor.matmul(ps[:], wsb[:, tap, :], rhs,
                                     start=(tap == 0), stop=(tap == 8))
                return ps

            def conv_act(idx, b, ch, ps):
                h0 = ch * CH
                dst = xp[idx + 1][:, b, 1 + h0:1 + h0 + CH, WL:WL + W]
                nc.scalar.activation(dst, ps[:], AF.Relu,
                                     bias=cbsb[:, idx:idx + 1], scale=1.0,
                                     accum_out=gsum[:, b, ch:ch + 1])

            def seq_add(b):
                dst = seq0[:, 0:1] if b == 0 else seq[:, b:b + 1]
                with nc.allow_low_precision("f32r rounding of f32 sum"):
                    nc.vector.tensor_tensor(dst, gsum[:, b, 0:1].bitcast(f32r),
                                            gsum[:, b, 1:2].bitcast(f32r),
                                            op=OP.add)

            def gates_post(pgt, nw, bias_i, bias_f, bias_c2, name):
                """sigmoid(i), sigmoid(f), sigmoid(2c+2bc) on [C, nw]."""
                si = wk.tile([C, max(nw, BL)], f32, tag=f"si{name}", name="si")
                sf = wk.tile([C, max(nw, BL)], f32, tag=f"sf{name}", name="sf")
                s2 = wk.tile([C, max(nw, BL)], f32, tag=f"s2{name}", name="s2")
                nc.scalar.activation(si[:, :nw], pgt[0][:, :nw], AF.Sigmoid,
                                     bias=bias_i)
                nc.scalar.activation(sf[:, :nw], pgt[1][:, :nw], AF.Sigmoid,
                                     bias=bias_f)
                nc.scalar.activation(s2[:, :nw], pgt[2][:, :nw], AF.Sigmoid,
                                     bias=bias_c2, scale=2.0)
                return si, sf, s2

            def gate_combine(si, sf, s2, cx_ap, nw, name):
                """ncx = sf*cx + si*(2*s2-1); returns [C, nw] f32 tile."""
                tmp = wk.tile([C, max(nw, BL)], f32, tag=f"tm{name}", name="tmp")
                nc.vector.scalar_tensor_tensor(tmp[:, :nw], s2[:, :nw], 2.0,
                                               si[:, :nw], op0=OP.mult,
                                               op1=OP.mult)
                ncx = wk.tile([C, max(nw, BL)], f32, tag=f"nx{name}", name="ncx")
                if cx_ap.free_size() == 1:
                    nc.vector.scalar_tensor_tensor(ncx[:, :nw], sf[:, :nw],
                                                   cx_ap, tmp[:, :nw],
                                                   op0=OP.mult, op1=OP.add)
                else:
                    t2 = wk.tile([C, max(nw, BL)], f32, tag=f"t2{name}",
                                 name="t2")
                    nc.vector.tensor_tensor(t2[:, :nw], sf[:, :nw], cx_ap,
                                            op=OP.mult)
                    nc.vector.tensor_tensor(ncx[:, :nw], t2[:, :nw], tmp[:, :nw],
                                            op=OP.add)
                nc.vector.tensor_tensor(ncx[:, :nw], ncx[:, :nw], si[:, :nw],
                                        op=OP.subtract)
                return ncx

            def dsu_update(sq, lo, hi, nw):
                """DSU over input columns [lo:hi) (hi-lo even); writes the
                first nw lanes' ht/ct columns [lo:lo+nw)."""
                w_ = hi - lo
                p1 = pu.tile([R, KV], f32, tag="u0", name="p1")
                nc.tensor.matmul(p1[:, :w_], w1sb[:, 0, :], sq[:, lo:hi],
                                 start=True, stop=True)
                p2 = pu.tile([R, KV], f32, tag="u1", name="p2")
                nc.tensor.matmul(p2[:, :w_], w1sb[:, 1, :], ht[:, lo:hi],
                                 start=True, stop=True)
                u1i = wk.tile([R, BLP], f32r, tag="u1i", name="u1i")
                nc.scalar.activation(u1i[:, :w_], p1[:, :w_], AF.Relu,
                                     bias=b1sb[:, 0:1], scale=inv_hw)
                u1h = wk.tile([R, BLP], f32r, tag="u1h", name="u1h")
                nc.scalar.activation(u1h[:, :w_], p2[:, :w_], AF.Relu,
                                     bias=b1sb[:, 1:2], scale=1.0)
                pgt = []
                for g in range(3):
                    pge = pg.tile([C, KV], f32, tag="g", name="pge")
                    nc.tensor.matmul(pge[:, :w_], w2sb[:, 0, g, :], u1i[:, :w_],
                                     start=True, stop=False)
                    nc.tensor.matmul(pge[:, :w_], w2sb[:, 1, g, :], u1h[:, :w_],
                                     start=False, stop=True)
                    pgt.append(pge)
                si, sf, s2 = gates_post(pgt, nw, b2tsb[:, 0:1],
                                        b2tsb[:, 1:2], b2tsb[:, 2:3], "x")
                ncx = gate_combine(si, sf, s2, ct[:, lo:lo + nw], nw, "x")
                nc.vector.tensor_copy(ct[:, lo:lo + nw], ncx[:, :nw])
                nc.scalar.activation(ht[:, lo:lo + nw], ct[:, lo:lo + nw],
                                     AF.Sigmoid, bias=0.0, scale=1.0)

            def softmax_group(idx, s0, s1):
                """qk -> scores -> sigmoid-ratio softmax -> aw for lanes
                [s0, s1)."""
                t, tp = idx, idx + (idx % 2)
                w_ = s1 - s0
                for b in range(s0, s1):
                    sqcol = seq0[:, 0:1] if b == 0 else seq[:, b:b + 1]
                    nc.vector.tensor_tensor(qk[:, b, :t], kr[:, b, :t],
                                            sqcol.broadcast_to([C, t]),
                                            op=OP.mult)
                psc = pg.tile([C, KV], f32, tag="g", name="psc")
                p3 = psc[:, :w_ * tp].rearrange("p (b t) -> p b t", b=w_)
                nc.tensor.matmul(p3[:], ones[:], qk[:, s0:s1, 0:tp],
                                 start=True, stop=True)
                nc.scalar.activation(sig[:, s0:s1, :t], p3[:, :, :t],
                                     AF.Sigmoid, bias=0.0, scale=exp_scale)
                one3 = ones[:, 0:w_ * t].bitcast(f32).rearrange(
                    "p (b t) -> p b t", b=w_)
                nc.vector.scalar_tensor_tensor(om[:, s0:s1, :t],
                                               sig[:, s0:s1, :t], -1.0,
                                               one3, op0=OP.mult, op1=OP.add)
                nc.vector.reciprocal(rr[:, s0:s1, :t], om[:, s0:s1, :t])
                nc.vector.tensor_tensor(ee[:, s0:s1, :t], sig[:, s0:s1, :t],
                                        rr[:, s0:s1, :t], op=OP.mult)
                nc.vector.tensor_reduce(den[:, s0:s1], ee[:, s0:s1, :t],
                                        axis=mybir.AxisListType.X, op=OP.add)
                nc.vector.reciprocal(dinv[:, s0:s1], den[:, s0:s1])
                for b in range(s0, s1):
                    nc.vector.tensor_tensor(om[:, b, :t], ee[:, b, :t],
                                            mm[:, b, :t], op=OP.mult)
                    nc.vector.tensor_tensor(
                        aw[:, b, :t], om[:, b, :t],
                        dinv[:, b:b + 1].broadcast_to([C, t]), op=OP.mult)


            def ctx_dve_chain(idx, b):
                t = idx
                dst = xp[idx + 1][:, b, 1:1 + H, WL:WL + W]
                for ti in range(t):
                    src = xp[ti + 1][:, b, 1:1 + H, WL:WL + W]
                    nc.vector.scalar_tensor_tensor(dst, src,
                                                   aw[:, b, ti:ti + 1], dst,
                                                   op0=OP.mult, op1=OP.add)

            def ctx_diags(idx, b, engines="AAP"):
                t = idx
                dgs = []
                for ti in range(t):
                    dg = dpool.tile([C, C], xdt, tag="diag", name="dg")
                    if engines[ti % len(engines)] == "P":
                        nc.gpsimd.tensor_tensor(
                            dg[:], eye[:],
                            aw[:, b, ti:ti + 1].broadcast_to([C, C]),
                            op=OP.mult)
                    else:
                        nc.scalar.mul(dg[:], eye[:], aw[:, b, ti:ti + 1])
                    dgs.append(dg)
                return dgs

            def ctx_pe_mm(idx, b, dgs):
                t = idx
                for ch in range(NCHUNK):
                    h0 = ch * CH
                    dst = xp[idx + 1][:, b, 1 + h0:1 + h0 + CH, WL:WL + W]
                    ps = pc.tile([C, CH, W], f32, tag="c", name="psx")
                    nc.tensor.matmul(ps[:], eye[:], dst, start=True, stop=False)
                    for k in range(t):
                        nc.tensor.matmul(ps[:], dgs[k][:],
                                         xp[k + 1][:, b, 1 + h0:1 + h0 + CH,
                                                   WL:WL + W],
                                         start=False, stop=(k == t - 1))
                    nc.scalar.activation(dst, ps[:], AF.Copy, bias=0.0,
                                         scale=1.0)

            def pr_update(idx):
                """pr slot for this layer via GAP linearity:
                pr[:, b, idx] = raw_gap(conv out) + sum_t aw*pr[:, b, t]."""
                t = idx
                nc.vector.tensor_tensor(rr[:, :, :t], aw[:, :, :t],
                                        pr[:, :, :t], op=OP.mult)
                nc.vector.tensor_reduce(den[:, :BL], rr[:, :, :t],
                                        axis=mybir.AxisListType.X, op=OP.add)
                nc.vector.tensor_tensor(pr[:, 0, idx:idx + 1], den[:, 0:1],
                                        seq0[:, 0:1].bitcast(f32), op=OP.add)
                nc.vector.tensor_tensor(pr[:, 1:5, idx], den[:, 1:5],
                                        seq[:, 1:5].bitcast(f32), op=OP.add)

            def emit_layer(idx, wsb, def_early, def_hook2):
                t = idx
                tp = t + (t % 2)
                last = idx == n_layers - 1
                first = idx == 0
                nk = BL * tp
                nkv = 2 * nk

                wsb_next = None
                if idx + 1 < n_layers:
                    wsb_next = wpool.tile([C, 9, C], xdt, tag="wsb",
                                          name="wsbn")
                    nc.sync.dma_start(wsb_next[:], CW[:, idx + 1])
                if 2 + idx < L:
                    zero_borders(idx + 2)   # borders needed by layer idx+1

                # ---- lane 0 conv ----
                conv_act(idx, 0, 0, conv_mm(idx, 0, 0, wsb))
                conv_act(idx, 0, 1, conv_mm(idx, 0, 1, wsb))
                seq_add(0)

                kv_k = kv_v = None
                if not first:
                    # kvin: [ k block | pooled*m block ], lanes 0-2 of the v
                    # block now; lanes 3-4 after the deferred chains below.
                    kv_k = kvin[:, :nk].rearrange("p (b t) -> p b t", b=BL)
                    kv_v = kvin[:, nk:nkv].rearrange("p (b t) -> p b t", b=BL)
                    nc.vector.tensor_copy(kv_k[:, :, :t], kr[:, :, :t])
                    nc.vector.tensor_tensor(kv_v[:, 0:3, :t], pr[:, 0:3, :t],
                                            mm[:, 0:3, :t], op=OP.mult)
                # deferred lane-3/4 context work from the previous layer
                for fn in def_early:
                    fn()
                if not first:
                    nc.vector.tensor_tensor(kv_v[:, 3:5, :t], pr[:, 3:5, :t],
                                            mm[:, 3:5, :t], op=OP.mult)

                # ---- lane 1 conv + dsu0 interleave ----
                conv_act(idx, 1, 0, conv_mm(idx, 1, 0, wsb))
                p1 = pu.tile([R, KV], f32, tag="u0", name="p1")
                nc.tensor.matmul(p1[:, :2], w1sb[:, 0, :], seq0[:, 0:2],
                                 start=True, stop=True)
                p2 = pu.tile([R, KV], f32, tag="u1", name="p2")
                nc.tensor.matmul(p2[:, :2], w1sb[:, 1, :], ht0[:, 0:2],
                                 start=True, stop=True)
                u1i = wk.tile([R, BLP], f32r, tag="u1i0", name="u1i")
                nc.scalar.activation(u1i[:, :2], p1[:, :2], AF.Relu,
                                     bias=b1sb[:, 0:1], scale=inv_hw)
                u1h = wk.tile([R, BLP], f32r, tag="u1h0", name="u1h")
                nc.scalar.activation(u1h[:, :2], p2[:, :2], AF.Relu,
                                     bias=b1sb[:, 1:2], scale=1.0)
                ps1b = conv_mm(idx, 1, 1, wsb)
                pgt = []
                for g in range(3):
                    pge = pg.tile([C, KV], f32, tag="g", name="pge")
                    nc.tensor.matmul(pge[:, :2], w2sb[:, 0, g, :], u1i[:, :2],
                                     start=True, stop=False)
                    nc.tensor.matmul(pge[:, :2], w2sb[:, 1, g, :], u1h[:, :2],
                                     start=False, stop=True)
                    pgt.append(pge)
                conv_act(idx, 1, 1, ps1b)
                seq_add(1)
                si, sf, s2 = gates_post(pgt, 1, b2tsb[:, 0:1], b2tsb[:, 1:2],
                                        b2tsb[:, 2:3], "x")
                ncx = gate_combine(si, sf, s2, ct0[:, 0:1], 1, "x")
                nc.vector.tensor_copy(ct0[:, 0:1], ncx[:, :1])
                nc.scalar.activation(ht0[:, 0:1], ct0[:, 0:1], AF.Sigmoid,
                                     bias=0.0, scale=1.0)
                cx = ct0[:, 0:1]

                # ---- lane 2 conv + cell_hh/kv-head interleave ----
                conv_act(idx, 2, 0, conv_mm(idx, 2, 0, wsb))
                pm1 = u1kv = None
                if not first:
                    pm1 = pu.tile([R, KV], f32, tag="u0", name="pm1")
                    nc.tensor.matmul(pm1[:, :nkv], w1sb[:, 0, :],
                                     kvin[:, :nkv], start=True, stop=True)
                    u1kv = wk.tile([R, KV], f32r, tag="u1kv", name="u1kv")
                    nc.scalar.activation(u1kv[:, :nkv], pm1[:, :nkv], AF.Relu,
                                         bias=b1sb[:, 0:1], scale=inv_hw)
                ps2b = conv_mm(idx, 2, 1, wsb)
                if not first:
                    ph1 = pu.tile([R, KV], f32, tag="u1", name="ph1")
                    nc.tensor.matmul(ph1[:, :2], w1sb[:, 1, :], ht0[:, 0:2],
                                     start=True, stop=True)
                    u1hx = wk.tile([R, 2], f32r, tag="u1hx", name="u1hx")
                    nc.scalar.activation(u1hx[:], ph1[:, :2], AF.Relu,
                                         bias=b1sb[:, 1:2], scale=1.0)
                conv_act(idx, 2, 1, ps2b)
                seq_add(2)

                # ---- lane 3 conv + kv gates interleave ----
                ps3a = conv_mm(idx, 3, 0, wsb)
                if not first:
                    biask = wk.tile([C, 3], f32, tag="biask", name="biask")
                    for g in range(3):
                        phg = pg.tile([C, KV], f32, tag="g", name="phg")
                        nc.tensor.matmul(phg[:, :2], w2sb[:, 1, g, :],
                                         u1hx[:], start=True, stop=True)
                        if g == 2:
                            nc.vector.scalar_tensor_tensor(
                                biask[:, 2:3], phg[:, 0:1], 2.0,
                                b2tsb[:, 2:3], op0=OP.mult, op1=OP.add)
                        else:
                            nc.vector.tensor_tensor(
                                biask[:, g:g + 1], phg[:, 0:1],
                                b2tsb[:, g:g + 1], op=OP.add)
                    pgt = []
                    for g in range(3):
                        pge = pg.tile([C, KV], f32, tag="g", name="pge")
                        nc.tensor.matmul(pge[:, :nkv], w2sb[:, 0, g, :],
                                         u1kv[:, :nkv], start=True, stop=True)
                        pgt.append(pge)
                conv_act(idx, 3, 0, ps3a)
                if not first:
                    si, sf, s2 = gates_post(pgt, nkv, biask[:, 0:1],
                                            biask[:, 1:2], biask[:, 2:3],
                                            "kv")
                    ncx = gate_combine(si, sf, s2, cx, nkv, "kv")
                    khvh = wk.tile([C, KV], f32, tag="khvh", name="khvh")
                    nc.scalar.activation(khvh[:, :nkv], ncx[:, :nkv],
                                         AF.Sigmoid, bias=0.0, scale=1.0)
                    kh = khvh[:, :nk].rearrange("p (b t) -> p b t", b=BL)
                    vh = khvh[:, nk:nkv].rearrange("p (b t) -> p b t", b=BL)
                    nc.vector.tensor_tensor(kr[:, :, :t], kr[:, :, :t],
                                            kh[:, :, :t], op=OP.mult)
                    nc.vector.tensor_tensor(mm[:, :, :t], mm[:, :, :t],
                                            vh[:, :, :t], op=OP.mult)
                for fn in def_hook2:
                    fn()
                conv_act(idx, 3, 1, conv_mm(idx, 3, 1, wsb))
                seq_add(3)

                if not first:
                    # lanes 0-2 softmax + lane1 context (DVE)
                    softmax_group(idx, 0, 3)
                    if not last:
                        ctx_dve_chain(idx, 1)

                # ---- lane 4 conv ----
                conv_act(idx, 4, 0, conv_mm(idx, 4, 0, wsb))
                dgs0 = None
                if not first and not last:
                    dgs0 = ctx_diags(idx, 0)   # Pool builds lane-0 diags
                conv_act(idx, 4, 1, conv_mm(idx, 4, 1, wsb))
                seq_add(4)

                if not first and last:
                    softmax_group(idx, 3, 5)
                    ctx_dve_chain(idx, 1)
                    nc.sync.dma_start(OUT[:, 1], xp[n_layers][:, 1])

                if not first:
                    ctx_dve_chain(idx, 2)
                    if last:
                        nc.sync.dma_start(OUT[:, 2], xp[n_layers][:, 2])
                    if not last:
                        ctx_pe_mm(idx, 0, dgs0)   # lane-0 ctx on PE

                if not first and not last:
                    softmax_group(idx, 3, 5)
                    pr_update(idx)

                if not last:
                    dsu_update(seq, 1, 5, 4)
                    # append this layer's q / modulation slot
                    nc.vector.tensor_copy(kr[:, 0, idx:idx + 1], seq0[:, 0:1])
                    nc.vector.tensor_copy(kr[:, 1:5, idx], seq[:, 1:5])
                    nc.gpsimd.memset(mm[:, :, idx], 1.0)
                    if first:
                        nc.vector.tensor_copy(pr[:, 0, idx:idx + 1],
                                              seq0[:, 0:1])
                        nc.vector.tensor_copy(pr[:, 1:5, idx], seq[:, 1:5])

                if not first:
                    if last:
                        # drain: lanes 3,4 on PE (diags Act / Pool in parallel)
                        dgs3 = ctx_diags(idx, 3, engines="A")
                        dgs4 = ctx_diags(idx, 4, engines="P")
                        ctx_pe_mm(idx, 3, dgs3)
                        nc.sync.dma_start(OUT[:, 3], xp[n_layers][:, 3])
                        ctx_pe_mm(idx, 4, dgs4)
                        nc.sync.dma_start(OUT[:, 4], xp[n_layers][:, 4])
                ne, nh = [], []
                if not first and not last:
                    def e3():
                        ctx_dve_chain(idx, 3)
                    ne.append(e3)
                    if idx <= 3:
                        def e4():
                            ctx_dve_chain(idx, 4)
                        ne.append(e4)
                    else:
                        cell = {}
                        def e4d():
                            cell["dgs"] = ctx_diags(idx, 4, engines="P")
                        ne.append(e4d)
                        def e4m():
                            ctx_pe_mm(idx, 4, cell["dgs"])
                        nh.append(e4m)
                return wsb_next, ne, nh

            wsb = wsb0
            de, dh = [], []
            for idx in range(n_layers):
                wsb, de, dh = emit_layer(idx, wsb, de, dh)
            if n_layers == 1:
                for b in range(1, BL):
                    nc.sync.dma_start(OUT[:, b], xp[1][:, b])

    nc.compile()
    return nc


def prep_inputs(x, conv_w, conv_b, ih_w1, ih_b1, ih_w2, ih_b2,
                hh_w1, hh_b1, hh_w2, hh_b2):
    """Host-side prep: pad/transpose into the kernel's layouts."""
    f = np.float32
    if KF32:
        xf = f
    else:
        import ml_dtypes
        xf = np.dtype(ml_dtypes.bfloat16)
    xt = np.ascontiguousarray(np.transpose(np.asarray(x, f), (1, 0, 2, 3)))
    cw = np.ascontiguousarray(
        np.transpose(np.asarray(conv_w, f), (2, 0, 3, 4, 1))
        .reshape(C, L, 9, C).astype(xf))
    cb = np.ascontiguousarray(np.asarray(conv_b, f).T)             # [C, L]
    w1 = np.ascontiguousarray(
        np.stack([np.asarray(ih_w1, f).T, np.asarray(hh_w1, f).T], axis=1))
    b1 = np.ascontiguousarray(
        np.stack([np.asarray(ih_b1, f), np.asarray(hh_b1, f)], axis=1))
    w2 = np.ascontiguousarray(
        np.stack([np.asarray(ih_w2, f).T.reshape(R, 3, C),
                  np.asarray(hh_w2, f).T.reshape(R, 3, C)], axis=1))
    b2t = (np.asarray(ih_b2, f) + np.asarray(hh_b2, f)).reshape(3, C).T.copy()
    b2t[:, 2] *= 2.0   # tanh-as-sigmoid trick needs the c bias pre-doubled
    ones = np.ones((C, C), f)
    eye = np.ascontiguousarray(np.eye(C, dtype=f).astype(xf))
    shards = []
    for k in range(NCORES):
        lanes = [0, 4 * k, 4 * k + 1, 4 * k + 2, 4 * k + 3]
        xs = np.zeros((C, BL, HP, WP), xf)
        xs[:, :, 1:1 + H, WL:WL + W] = xt[:, lanes].astype(xf)
        shards.append(np.ascontiguousarray(xs))
    common = {"cw": cw, "cb": cb, "w1": w1, "b1": b1, "w2": w2,
              "b2t": np.ascontiguousarray(b2t), "ones": ones, "eye": eye}
    return [dict(common, x0=shards[k]) for k in range(NCORES)]


def gather_out(results):
    """results: list of per-core dicts with 'out' [C, BL, HP, WP]."""
    out = np.empty((B, C, H, W), np.float32)
    for k in range(NCORES):
        o = np.asarray(results[k]["out"][:, 1:5, 1:1 + H, WL:WL + W],
                       np.float32)
        out[4 * k:4 * k + 4] = np.transpose(o, (1, 0, 2, 3))
    return out


_NC_CACHE = {}


def kernel(**inputs) -> np.ndarray:
    from concourse.bass_utils import run_bass_kernel_spmd

    if "nc" not in _NC_CACHE:
        _NC_CACHE["nc"] = build_nc()
    nc = _NC_CACHE["nc"]
    in_maps = prep_inputs(**inputs)
    res = run_bass_kernel_spmd(nc, in_maps, core_ids=list(range(NCORES)))
    return gather_out(res.results)


if __name__ == "__main__":
    nc = build_nc()
    print("built ok")


# revision 15
# speedup vs baseline: 1.2926x; 1.0160x over previous
"""Trainium2 Bass kernel for nn_Attention_40346922778795 (v2 schedule).

8 layers of: conv3x3+ReLU -> GAP -> DSU recurrence; k/v modulation driven by
batch-0 hidden state; additive attention over layer history.

Distribution: batch data-parallel, 4 owned batches + 1 replicated batch-0
lane per core (hidden[0][0] dependency stays local; zero collectives).

v2 schedule changes vs v1 baseline (301us):
 - conv activations write straight into the next xp buffer; the attention
   context accumulates IN-PLACE there (no xc staging tile).
 - bf16 feature maps by default (KF32=1 env falls back to fp32r): DVE AXPYs
   run in 2x packed mode, LDWEIGHTS halves, DMA halves.
 - softmax exp computed as sigmoid ratio e^s = p/(1-p), keeping ScalarE in
   the sigmoid activation table all kernel long (no ACT_TABLE_LOAD thrash).
 - DSU / cell_hh / kv-gate matmul chains interleaved INTO the conv matmul
   stream so TensorE never idles waiting on them and stays at high p-state.
 - per-lane context routing: lane0 on TensorE (diag matmuls, feeds next
   layer's first conv ASAP), lanes 1-2 DVE inline, lanes 3-4 DVE deferred
   into the next layer's stream (their convs run late, so there is slack).
 - last layer: lane-0 context + store skipped (replica lane is dead there),
   lane4 context on TensorE to shorten the DVE drain.
"""

import os
import sys

for _p in ("/opt/trn_rl_repo",):
    if _p not in sys.path and os.path.isdir(_p):
        sys.path.insert(0, _p)
os.environ.setdefault("MYCRO_LOCAL_CACHE", "1")

import numpy as np  # noqa: E402

B, C, H, W, L = 32, 128, 28, 28, 8
R = C // 4
BL = 5          # lanes per core: [batch0 replica, 4 owned batches]
KF32 = bool(os.environ.get("KF32"))
HP = H + 2
# bf16 x-buffers use left-pad 2 so every 28-wide interior row starts on a
# 4-byte boundary (DVE 2x packed mode requirement); fp32r keeps pad 1.
WL = 1 if KF32 else 2
WP = 30 if KF32 else 32
NCORES = 8
CH = 14         # conv chunk rows
BLP = 6         # lane count padded to even (fp32r matmul ISA restriction)
NCHUNK = H // CH


def build_nc(n_layers=L):
    import concourse.bacc as bacc
    import concourse.mybir as mybir
    import concourse.tile as tile

    dt = mybir.dt
    AF = mybir.ActivationFunctionType
    OP = mybir.AluOpType
    f32, f32r = dt.float32, dt.float32r
    xdt = f32r if KF32 else dt.bfloat16

    nc = bacc.Bacc()

    X0 = nc.dram_tensor("x0", [C, BL, HP, WP], xdt, kind="ExternalInput")
    CW = nc.dram_tensor("cw", [C, L, 9, C], xdt, kind="ExternalInput")
    CB = nc.dram_tensor("cb", [C, L], f32, kind="ExternalInput")
    W1 = nc.dram_tensor("w1", [C, 2, R], f32r, kind="ExternalInput")
    B1 = nc.dram_tensor("b1", [R, 2], f32, kind="ExternalInput")
    W2 = nc.dram_tensor("w2", [R, 2, 3, C], f32r, kind="ExternalInput")
    # b2t columns: [i, f, 2*c] (c-gate bias pre-doubled for the tanh trick)
    B2T = nc.dram_tensor("b2t", [C, 3], f32, kind="ExternalInput")
    ONES = nc.dram_tensor("ones", [C, C], f32r, kind="ExternalInput")
    EYE = nc.dram_tensor("eye", [C, C], xdt, kind="ExternalInput")
    OUT = nc.dram_tensor("out", [C, BL, HP, WP], xdt, kind="ExternalOutput")

    inv_hw = 1.0 / (H * W)
    exp_scale = 1.0 / (float(H * W) ** 2 * np.sqrt(np.float32(C)))
    KV = 2 * L * BL   # width of the merged k/v modulation block

    with tile.TileContext(nc) as tc:
        with tc.tile_pool(name="xpad", bufs=1) as xpool, \
             tc.tile_pool(name="wts", bufs=2) as wpool, \
             tc.tile_pool(name="small", bufs=1) as spool, \
             tc.tile_pool(name="work", bufs=2) as wk, \
             tc.tile_pool(name="dpool", bufs=8) as dpool, \
             tc.tile_pool(name="pc", bufs=3, space="PSUM") as pc, \
             tc.tile_pool(name="pg", bufs=3, space="PSUM") as pg, \
             tc.tile_pool(name="pu", bufs=1, space="PSUM") as pu:

            # ---- persistent state (attention tensors are [C, BL, L]) ----
            xp = [xpool.tile([C, BL, HP, WP], xdt, tag=f"xp{i}", name=f"xp{i}")
                  for i in range(L)]
            xp.append(xp[0])  # layer-8 output reuses the input buffer
            kr = spool.tile([C, BL, L], f32r, tag="kr")     # raw q sums, modulated
            mm = spool.tile([C, BL, L], f32, tag="mm")      # running v modulation
            pr = spool.tile([C, BL, L], f32, tag="pr")      # raw pooled-v sums
            w1sb = spool.tile([C, 2, R], f32r, tag="w1sb")
            b1sb = spool.tile([R, 2], f32, tag="b1sb")
            w2sb = spool.tile([R, 2, 3, C], f32r, tag="w2sb")
            b2tsb = spool.tile([C, 3], f32, tag="b2t")
            cbsb = spool.tile([C, L], f32, tag="cbsb")
            ones = spool.tile([C, C], f32r, tag="ones")
            eye = spool.tile([C, C], xdt, tag="eye")
            ht = spool.tile([C, BLP], f32r, tag="ht")
            ct = spool.tile([C, BL], f32, tag="ct")
            ht0 = spool.tile([C, 2], f32r, tag="ht0")
            ct0 = spool.tile([C, 1], f32, tag="ct0")
            seq = spool.tile([C, BLP], f32r, tag="seq")
            seq0 = spool.tile([C, 2], f32r, tag="seq0")
            kvin = spool.tile([C, KV], f32r, tag="kvin")
            qk = spool.tile([C, BL, L], f32r, tag="qk")
            sig = spool.tile([C, BL, L], f32, tag="sig")
            om = spool.tile([C, BL, L], f32, tag="om")
            rr = spool.tile([C, BL, L], f32, tag="rr")
            ee = spool.tile([C, BL, L], f32, tag="ee")
            den = spool.tile([C, BL], f32, tag="den")
            dinv = spool.tile([C, BL], f32, tag="dinv")
            aw = spool.tile([C, BL, L], f32, tag="aw")
            gsum = spool.tile([C, BL, NCHUNK], f32, tag="gsum")
            pacc = spool.tile([C, BL, NCHUNK], f32, tag="pacc")

            # ---- one-time loads & init ----
            wsb0 = wpool.tile([C, 9, C], xdt, tag="wsb", name="wsb0")
            nc.sync.dma_start(wsb0[:], CW[:, 0])
            for b in range(BL):
                nc.sync.dma_start(xp[0][:, b], X0[:, b])
            nc.sync.dma_start(w1sb[:], W1[:])
            nc.sync.dma_start(b1sb[:], B1[:])
            nc.sync.dma_start(w2sb[:], W2[:])
            nc.sync.dma_start(b2tsb[:], B2T[:])
            nc.sync.dma_start(cbsb[:], CB[:])
            nc.sync.dma_start(ones[:], ONES[:])
            nc.sync.dma_start(eye[:], EYE[:])
            nc.gpsimd.memset(ht[:].bitcast(f32), 0.0)
            nc.gpsimd.memset(ct[:], 0.0)
            nc.gpsimd.memset(ht0[:].bitcast(f32), 0.0)
            nc.gpsimd.memset(ct0[:], 0.0)
            nc.gpsimd.memset(seq[:].bitcast(f32), 0.0)
            nc.gpsimd.memset(seq0[:].bitcast(f32), 0.0)
            nc.gpsimd.memset(kvin[:].bitcast(f32), 0.0)
            nc.gpsimd.memset(kr[:].bitcast(f32), 0.0)
            nc.gpsimd.memset(qk[:].bitcast(f32), 0.0)
            nc.gpsimd.memset(mm[:], 0.0)
            nc.gpsimd.memset(pr[:], 0.0)
            # pad borders of buffers 1..7 are zeroed lazily, one buffer per
            # layer, inside emit_layer (keeps Pool off the startup critical
            # path; buffer 8 aliases the host-zero-padded input buffer)
            bdt = f32 if KF32 else dt.uint16

            def zero_borders(i):
                nc.gpsimd.memset(xp[i][:, :, 0, :].bitcast(bdt), 0)
                nc.gpsimd.memset(xp[i][:, :, HP - 1, :].bitcast(bdt), 0)
                nc.gpsimd.memset(xp[i][:, :, 1:HP - 1, 0:WL].bitcast(bdt), 0)
                nc.gpsimd.memset(xp[i][:, :, 1:HP - 1, WL + W:WP].bitcast(bdt), 0)
            zero_borders(1)

            def conv_mm(idx, b, ch, wsb):
                h0 = ch * CH
                ps = pc.tile([C, CH, W], f32, tag="c", name="ps")
                for tap in range(9):
                    dy, dx = tap // 3, tap % 3
                    rhs = xp[idx][:, b, h0 + dy:h0 + dy + CH,
                                  dx + WL - 1:dx + WL - 1 + W]
                    nc.tensor.matmul(ps[:], wsb[:, tap, :], rhs,
                                     start=(tap == 0), stop=(tap == 8))
                return ps

            def conv_act(idx, b, ch, ps):
                h0 = ch * CH
                dst = xp[idx + 1][:, b, 1 + h0:1 + h0 + CH, WL:WL + W]
                nc.scalar.activation(dst, ps[:], AF.Relu,
                                     bias=cbsb[:, idx:idx + 1], scale=1.0,
                                     accum_out=gsum[:, b, ch:ch + 1])

            def seq_add(b):
                dst = seq0[:, 0:1] if b == 0 else seq[:, b:b + 1]
                with nc.allow_low_precision("f32r rounding of f32 sum"):
                    nc.vector.tensor_tensor(dst, gsum[:, b, 0:1].bitcast(f32r),
                                            gsum[:, b, 1:2].bitcast(f32r),
                                            op=OP.add)

            def gates_post(pgt, nw, bias_i, bias_f, bias_c2, name):
                """sigmoid(i), sigmoid(f), sigmoid(2c+2bc) on [C, nw]."""
                si = wk.tile([C, max(nw, BL)], f32, tag=f"si{name}", name="si")
                sf = wk.tile([C, max(nw, BL)], f32, tag=f"sf{name}", name="sf")
                s2 = wk.tile([C, max(nw, BL)], f32, tag=f"s2{name}", name="s2")
                nc.scalar.activation(si[:, :nw], pgt[0][:, :nw], AF.Sigmoid,
                                     bias=bias_i)
                nc.scalar.activation(sf[:, :nw], pgt[1][:, :nw], AF.Sigmoid,
                                     bias=bias_f)
                nc.scalar.activation(s2[:, :nw], pgt[2][:, :nw], AF.Sigmoid,
                                     bias=bias_c2, scale=2.0)
                return si, sf, s2

            def gate_combine(si, sf, s2, cx_ap, nw, name):
                """ncx = sf*cx + si*(2*s2-1); returns [C, nw] f32 tile."""
                tmp = wk.tile([C, max(nw, BL)], f32, tag=f"tm{name}", name="tmp")
                nc.vector.scalar_tensor_tensor(tmp[:, :nw], s2[:, :nw], 2.0,
                                               si[:, :nw], op0=OP.mult,
                                               op1=OP.mult)
                ncx = wk.tile([C, max(nw, BL)], f32, tag=f"nx{name}", name="ncx")
                if cx_ap.free_size() == 1:
                    nc.vector.scalar_tensor_tensor(ncx[:, :nw], sf[:, :nw],
                                                   cx_ap, tmp[:, :nw],
                                                   op0=OP.mult, op1=OP.add)
                else:
                    t2 = wk.tile([C, max(nw, BL)], f32, tag=f"t2{name}",
                                 name="t2")
                    nc.vector.tensor_tensor(t2[:, :nw], sf[:, :nw], cx_ap,
                                            op=OP.mult)
                    nc.vector.tensor_tensor(ncx[:, :nw], t2[:, :nw], tmp[:, :nw],
                                            op=OP.add)
                nc.vector.tensor_tensor(ncx[:, :nw], ncx[:, :nw], si[:, :nw],
                                        op=OP.subtract)
                return ncx

            def dsu_update(sq, lo, hi, nw):
                """DSU over input columns [lo:hi) (hi-lo even); writes the
                first nw lanes' ht/ct columns [lo:lo+nw)."""
                w_ = hi - lo
                p1 = pu.tile([R, KV], f32, tag="u0", name="p1")
                nc.tensor.matmul(p1[:, :w_], w1sb[:, 0, :], sq[:, lo:hi],
                                 start=True, stop=True)
                p2 = pu.tile([R, KV], f32, tag="u1", name="p2")
                nc.tensor.matmul(p2[:, :w_], w1sb[:, 1, :], ht[:, lo:hi],
                                 start=True, stop=True)
                u1i = wk.tile([R, BLP], f32r, tag="u1i", name="u1i")
                nc.scalar.activation(u1i[:, :w_], p1[:, :w_], AF.Relu,
                                     bias=b1sb[:, 0:1], scale=inv_hw)
                u1h = wk.tile([R, BLP], f32r, tag="u1h", name="u1h")
                nc.scalar.activation(u1h[:, :w_], p2[:, :w_], AF.Relu,
                                     bias=b1sb[:, 1:2], scale=1.0)
                pgt = []
                for g in range(3):
                    pge = pg.tile([C, KV], f32, tag="g", name="pge")
                    nc.tensor.matmul(pge[:, :w_], w2sb[:, 0, g, :], u1i[:, :w_],
                                     start=True, stop=False)
                    nc.tensor.matmul(pge[:, :w_], w2sb[:, 1, g, :], u1h[:, :w_],
                                     start=False, stop=True)
                    pgt.append(pge)
                si, sf, s2 = gates_post(pgt, nw, b2tsb[:, 0:1],
                                        b2tsb[:, 1:2], b2tsb[:, 2:3], "x")
                ncx = gate_combine(si, sf, s2, ct[:, lo:lo + nw], nw, "x")
                nc.vector.tensor_copy(ct[:, lo:lo + nw], ncx[:, :nw])
                nc.scalar.activation(ht[:, lo:lo + nw], ct[:, lo:lo + nw],
                                     AF.Sigmoid, bias=0.0, scale=1.0)

            def softmax_group(idx, s0, s1):
                """qk -> scores -> sigmoid-ratio softmax -> aw for lanes
                [s0, s1)."""
                t, tp = idx, idx + (idx % 2)
                w_ = s1 - s0
                for b in range(s0, s1):
                    sqcol = seq0[:, 0:1] if b == 0 else seq[:, b:b + 1]
                    nc.vector.tensor_tensor(qk[:, b, :t], kr[:, b, :t],
                                            sqcol.broadcast_to([C, t]),
                                            op=OP.mult)
                psc = pg.tile([C, KV], f32, tag="g", name="psc")
                p3 = psc[:, :w_ * tp].rearrange("p (b t) -> p b t", b=w_)
                nc.tensor.matmul(p3[:], ones[:], qk[:, s0:s1, 0:tp],
                                 start=True, stop=True)
                nc.scalar.activation(sig[:, s0:s1, :t], p3[:, :, :t],
                                     AF.Sigmoid, bias=0.0, scale=exp_scale)
                one3 = ones[:, 0:w_ * t].bitcast(f32).rearrange(
                    "p (b t) -> p b t", b=w_)
                nc.vector.scalar_tensor_tensor(om[:, s0:s1, :t],
                                               sig[:, s0:s1, :t], -1.0,
                                               one3, op0=OP.mult, op1=OP.add)
                nc.vector.reciprocal(rr[:, s0:s1, :t], om[:, s0:s1, :t])
                nc.vector.tensor_tensor(ee[:, s0:s1, :t], sig[:, s0:s1, :t],
                                        rr[:, s0:s1, :t], op=OP.mult)
                nc.vector.tensor_reduce(den[:, s0:s1], ee[:, s0:s1, :t],
                                        axis=mybir.AxisListType.X, op=OP.add)
                nc.vector.reciprocal(dinv[:, s0:s1], den[:, s0:s1])
                for b in range(s0, s1):
                    nc.vector.tensor_tensor(om[:, b, :t], ee[:, b, :t],
                                            mm[:, b, :t], op=OP.mult)
                    nc.vector.tensor_tensor(
                        aw[:, b, :t], om[:, b, :t],
                        dinv[:, b:b + 1].broadcast_to([C, t]), op=OP.mult)


            def ctx_dve_chain(idx, b):
                t = idx
                dst = xp[idx + 1][:, b, 1:1 + H, WL:WL + W]
                for ti in range(t):
                    src = xp[ti + 1][:, b, 1:1 + H, WL:WL + W]
                    nc.vector.scalar_tensor_tensor(dst, src,
                                                   aw[:, b, ti:ti + 1], dst,
                                                   op0=OP.mult, op1=OP.add)

            def ctx_diags(idx, b, engines="AAP"):
                t = idx
                dgs = []
                for ti in range(t):
                    dg = dpool.tile([C, C], xdt, tag="diag", name="dg")
                    if engines[ti % len(engines)] == "P":
                        nc.gpsimd.tensor_tensor(
                            dg[:], eye[:],
                            aw[:, b, ti:ti + 1].broadcast_to([C, C]),
                            op=OP.mult)
                    else:
                        nc.scalar.mul(dg[:], eye[:], aw[:, b, ti:ti + 1])
                    dgs.append(dg)
                return dgs

            def ctx_pe_mm(idx, b, dgs):
                t = idx
                for ch in range(NCHUNK):
                    h0 = ch * CH
                    dst = xp[idx + 1][:, b, 1 + h0:1 + h0 + CH, WL:WL + W]
                    ps = pc.tile([C, CH, W], f32, tag="c", name="psx")
                    nc.tensor.matmul(ps[:], eye[:], dst, start=True, stop=False)
                    for k in range(t):
                        nc.tensor.matmul(ps[:], dgs[k][:],
                                         xp[k + 1][:, b, 1 + h0:1 + h0 + CH,
                                                   WL:WL + W],
                                         start=False, stop=(k == t - 1))
                    nc.scalar.activation(dst, ps[:], AF.Copy, bias=0.0,
                                         scale=1.0)

            def pr_update(idx):
                """pr slot for this layer via GAP linearity:
                pr[:, b, idx] = raw_gap(conv out) + sum_t aw*pr[:, b, t]."""
                t = idx
                nc.vector.tensor_tensor(rr[:, :, :t], aw[:, :, :t],
                                        pr[:, :, :t], op=OP.mult)
                nc.vector.tensor_reduce(den[:, :BL], rr[:, :, :t],
                                        axis=mybir.AxisListType.X, op=OP.add)
                nc.vector.tensor_tensor(pr[:, 0, idx:idx + 1], den[:, 0:1],
                                        seq0[:, 0:1].bitcast(f32), op=OP.add)
                nc.vector.tensor_tensor(pr[:, 1:5, idx], den[:, 1:5],
                                        seq[:, 1:5].bitcast(f32), op=OP.add)

            def emit_layer(idx, wsb, def_early, def_hook2):
                t = idx
                tp = t + (t % 2)
                last = idx == n_layers - 1
                first = idx == 0
                nk = BL * tp
                nkv = 2 * nk

                wsb_next = None
                if idx + 1 < n_layers:
                    wsb_next = wpool.tile([C, 9, C], xdt, tag="wsb",
                                          name="wsbn")
                    nc.sync.dma_start(wsb_next[:], CW[:, idx + 1])
                if 2 + idx < L:
                    zero_borders(idx + 2)   # borders needed by layer idx+1

                # ---- lane 0 conv ----
                conv_act(idx, 0, 0, conv_mm(idx, 0, 0, wsb))
                conv_act(idx, 0, 1, conv_mm(idx, 0, 1, wsb))
                seq_add(0)

                kv_k = kv_v = None
                if not first:
                    # kvin: [ k block | pooled*m block ], lanes 0-2 of the v
                    # block now; lanes 3-4 after the deferred chains below.
                    kv_k = kvin[:, :nk].rearrange("p (b t) -> p b t", b=BL)
                    kv_v = kvin[:, nk:nkv].rearrange("p (b t) -> p b t", b=BL)
                    nc.vector.tensor_copy(kv_k[:, :, :t], kr[:, :, :t])
                    nc.vector.tensor_tensor(kv_v[:, 0:3, :t], pr[:, 0:3, :t],
                                            mm[:, 0:3, :t], op=OP.mult)
                # deferred lane-3/4 context work from the previous layer
                for fn in def_early:
                    fn()
                if not first:
                    nc.vector.tensor_tensor(kv_v[:, 3:5, :t], pr[:, 3:5, :t],
                                            mm[:, 3:5, :t], op=OP.mult)

                # ---- lane 1 conv + dsu0 interleave ----
                conv_act(idx, 1, 0, conv_mm(idx, 1, 0, wsb))
                p1 = pu.tile([R, KV], f32, tag="u0", name="p1")
                nc.tensor.matmul(p1[:, :2], w1sb[:, 0, :], seq0[:, 0:2],
                                 start=True, stop=True)
                p2 = pu.tile([R, KV], f32, tag="u1", name="p2")
                nc.tensor.matmul(p2[:, :2], w1sb[:, 1, :], ht0[:, 0:2],
                                 start=True, stop=True)
                u1i = wk.tile([R, BLP], f32r, tag="u1i0", name="u1i")
                nc.scalar.activation(u1i[:, :2], p1[:, :2], AF.Relu,
                                     bias=b1sb[:, 0:1], scale=inv_hw)
                u1h = wk.tile([R, BLP], f32r, tag="u1h0", name="u1h")
                nc.scalar.activation(u1h[:, :2], p2[:, :2], AF.Relu,
                                     bias=b1sb[:, 1:2], scale=1.0)
                ps1b = conv_mm(idx, 1, 1, wsb)
                pgt = []
                for g in range(3):
                    pge = pg.tile([C, KV], f32, tag="g", name="pge")
                    nc.tensor.matmul(pge[:, :2], w2sb[:, 0, g, :], u1i[:, :2],
                                     start=True, stop=False)
                    nc.tensor.matmul(pge[:, :2], w2sb[:, 1, g, :], u1h[:, :2],
                                     start=False, stop=True)
                    pgt.append(pge)
                conv_act(idx, 1, 1, ps1b)
                seq_add(1)
                si, sf, s2 = gates_post(pgt, 1, b2tsb[:, 0:1], b2tsb[:, 1:2],
                                        b2tsb[:, 2:3], "x")
                ncx = gate_combine(si, sf, s2, ct0[:, 0:1], 1, "x")
                nc.vector.tensor_copy(ct0[:, 0:1], ncx[:, :1])
                nc.scalar.activation(ht0[:, 0:1], ct0[:, 0:1], AF.Sigmoid,
                                     bias=0.0, scale=1.0)
                cx = ct0[:, 0:1]

                # ---- lane 2 conv + cell_hh/kv-head interleave ----
                conv_act(idx, 2, 0, conv_mm(idx, 2, 0, wsb))
                pm1 = u1kv = None
                if not first:
                    pm1 = pu.tile([R, KV], f32, tag="u0", name="pm1")
                    nc.tensor.matmul(pm1[:, :nkv], w1sb[:, 0, :],
                                     kvin[:, :nkv], start=True, stop=True)
                    u1kv = wk.tile([R, KV], f32r, tag="u1kv", name="u1kv")
                    nc.scalar.activation(u1kv[:, :nkv], pm1[:, :nkv], AF.Relu,
                                         bias=b1sb[:, 0:1], scale=inv_hw)
                ps2b = conv_mm(idx, 2, 1, wsb)
                if not first:
                    ph1 = pu.tile([R, KV], f32, tag="u1", name="ph1")
                    nc.tensor.matmul(ph1[:, :2], w1sb[:, 1, :], ht0[:, 0:2],
                                     start=True, stop=True)
                    u1hx = wk.tile([R, 2], f32r, tag="u1hx", name="u1hx")
                    nc.scalar.activation(u1hx[:], ph1[:, :2], AF.Relu,
                                         bias=b1sb[:, 1:2], scale=1.0)
                conv_act(idx, 2, 1, ps2b)
                seq_add(2)

                # ---- lane 3 conv + kv gates interleave ----
                ps3a = conv_mm(idx, 3, 0, wsb)
                if not first:
                    biask = wk.tile([C, 3], f32, tag="biask", name="biask")
                    for g in range(3):
                        phg = pg.tile([C, KV], f32, tag="g", name="phg")
                        nc.tensor.matmul(phg[:, :2], w2sb[:, 1, g, :],
                                         u1hx[:], start=True, stop=True)
                        if g == 2:
                            nc.vector.scalar_tensor_tensor(
                                biask[:, 2:3], phg[:, 0:1], 2.0,
                                b2tsb[:, 2:3], op0=OP.mult, op1=OP.add)
                        else:
                            nc.vector.tensor_tensor(
                                biask[:, g:g + 1], phg[:, 0:1],
                                b2tsb[:, g:g + 1], op=OP.add)
                    pgt = []
                    for g in range(3):
                        pge = pg.tile([C, KV], f32, tag="g", name="pge")
                        nc.tensor.matmul(pge[:, :nkv], w2sb[:, 0, g, :],
                                         u1kv[:, :nkv], start=True, stop=True)
                        pgt.append(pge)
                conv_act(idx, 3, 0, ps3a)
                if not first:
                    si, sf, s2 = gates_post(pgt, nkv, biask[:, 0:1],
                                            biask[:, 1:2], biask[:, 2:3],
                                            "kv")
                    ncx = gate_combine(si, sf, s2, cx, nkv, "kv")
                    khvh = wk.tile([C, KV], f32, tag="khvh", name="khvh")
                    nc.scalar.activation(khvh[:, :nkv], ncx[:, :nkv],
                                         AF.Sigmoid, bias=0.0, scale=1.0)
                    kh = khvh[:, :nk].rearrange("p (b t) -> p b t", b=BL)
                    vh = khvh[:, nk:nkv].rearrange("p (b t) -> p b t", b=BL)
                    nc.vector.tensor_tensor(kr[:, :, :t], kr[:, :, :t],
                                            kh[:, :, :t], op=OP.mult)
                    nc.vector.tensor_tensor(mm[:, :, :t], mm[:, :, :t],
                                            vh[:, :, :t], op=OP.mult)
                for fn in def_hook2:
                    fn()
                conv_act(idx, 3, 1, conv_mm(idx, 3, 1, wsb))
                seq_add(3)

                if not first:
                    # lanes 0-2 softmax + lane1 context (DVE)
                    softmax_group(idx, 0, 3)
                    if not last:
                        ctx_dve_chain(idx, 1)

                # ---- lane 4 conv ----
                conv_act(idx, 4, 0, conv_mm(idx, 4, 0, wsb))
                dgs0 = None
                if not first and not last:
                    dgs0 = ctx_diags(idx, 0)   # Pool builds lane-0 diags
                conv_act(idx, 4, 1, conv_mm(idx, 4, 1, wsb))
                seq_add(4)

                if not first and last:
                    softmax_group(idx, 3, 5)
                    ctx_dve_chain(idx, 1)
                    nc.sync.dma_start(OUT[:, 1], xp[n_layers][:, 1])

                if not first:
                    ctx_dve_chain(idx, 2)
                    if last:
                        nc.sync.dma_start(OUT[:, 2], xp[n_layers][:, 2])
                    if not last:
                        ctx_pe_mm(idx, 0, dgs0)   # lane-0 ctx on PE

                if not first and not last:
                    softmax_group(idx, 3, 5)
                    pr_update(idx)

                if not last:
                    # append this layer's q / modulation slot
                    nc.vector.tensor_copy(kr[:, 0, idx:idx + 1], seq0[:, 0:1])
                    nc.vector.tensor_copy(kr[:, 1:5, idx], seq[:, 1:5])
                    nc.gpsimd.memset(mm[:, :, idx], 1.0)
                    dsu_update(seq, 1, 5, 4)
                    if first:
                        nc.vector.tensor_copy(pr[:, 0, idx:idx + 1],
                                              seq0[:, 0:1])
                        nc.vector.tensor_copy(pr[:, 1:5, idx], seq[:, 1:5])

                if not first:
                    if last:
                        # drain: lanes 3,4 on PE (diags Act / Pool in parallel)
                        dgs3 = ctx_diags(idx, 3, engines="A")
                        dgs4 = ctx_diags(idx, 4, engines="P")
                        ctx_pe_mm(idx, 3, dgs3)
                        nc.sync.dma_start(OUT[:, 3], xp[n_layers][:, 3])
                        ctx_pe_mm(idx, 4, dgs4)
                        nc.sync.dma_start(OUT[:, 4], xp[n_layers][:, 4])
                ne, nh = [], []
                if not first and not last:
                    def e3():
                        ctx_dve_chain(idx, 3)
                    ne.append(e3)
                    if idx <= 3:
                        def e4():
                            ctx_dve_chain(idx, 4)
                        ne.append(e4)
                    else:
                        cell = {}
                        def e4d():
                            cell["dgs"] = ctx_diags(idx, 4, engines="P")
                        ne.append(e4d)
                        def e4m():
                            ctx_pe_mm(idx, 4, cell["dgs"])
                        nh.append(e4m)
                return wsb_next, ne, nh

            wsb = wsb0
            de, dh = [], []
            for idx in range(n_layers):
                wsb, de, dh = emit_layer(idx, wsb, de, dh)
            if n_layers == 1:
                for b in range(1, BL):
                    nc.sync.dma_start(OUT[:, b], xp[1][:, b])

    nc.compile()
    return nc


def prep_inputs(x, conv_w, conv_b, ih_w1, ih_b1, ih_w2, ih_b2,
                hh_w1, hh_b1, hh_w2, hh_b2):
    """Host-side prep: pad/transpose into the kernel's layouts."""
    f = np.float32
    if KF32:
        xf = f
    else:
        import ml_dtypes
        xf = np.dtype(ml_dtypes.bfloat16)
    xt = np.ascontiguousarray(np.transpose(np.asarray(x, f), (1, 0, 2, 3)))
    cw = np.ascontiguousarray(
        np.transpose(np.asarray(conv_w, f), (2, 0, 3, 4, 1))
        .reshape(C, L, 9, C).astype(xf))
    cb = np.ascontiguousarray(np.asarray(conv_b, f).T)             # [C, L]
    w1 = np.ascontiguousarray(
        np.stack([np.asarray(ih_w1, f).T, np.asarray(hh_w1, f).T], axis=1))
    b1 = np.ascontiguousarray(
        np.stack([np.asarray(ih_b1, f), np.asarray(hh_b1, f)], axis=1))
    w2 = np.ascontiguousarray(
        np.stack([np.asarray(ih_w2, f).T.reshape(R, 3, C),
                  np.asarray(hh_w2, f).T.reshape(R, 3, C)], axis=1))
    b2t = (np.asarray(ih_b2, f) + np.asarray(hh_b2, f)).reshape(3, C).T.copy()
    b2t[:, 2] *= 2.0   # tanh-as-sigmoid trick needs the c bias pre-doubled
    ones = np.ones((C, C), f)
    eye = np.ascontiguousarray(np.eye(C, dtype=f).astype(xf))
    shards = []
    for k in range(NCORES):
        lanes = [0, 4 * k, 4 * k + 1, 4 * k + 2, 4 * k + 3]
        xs = np.zeros((C, BL, HP, WP), xf)
        xs[:, :, 1:1 + H, WL:WL + W] = xt[:, lanes].astype(xf)
        shards.append(np.ascontiguousarray(xs))
    common = {"cw": cw, "cb": cb, "w1": w1, "b1": b1, "w2": w2,
              "b2t": np.ascontiguousarray(b2t), "ones": ones, "eye": eye}
    return [dict(common, x0=shards[k]) for k in range(NCORES)]


def gather_out(results):
    """results: list of per-core dicts with 'out' [C, BL, HP, WP]."""
    out = np.empty((B, C, H, W), np.float32)
    for k in range(NCORES):
        o = np.asarray(results[k]["out"][:, 1:5, 1:1 + H, WL:WL + W],
                       np.float32)
        out[4 * k:4 * k + 4] = np.transpose(o, (1, 0, 2, 3))
    return out


_NC_CACHE = {}


def kernel(**inputs) -> np.ndarray:
    from concourse.bass_utils import run_bass_kernel_spmd

    if "nc" not in _NC_CACHE:
        _NC_CACHE["nc"] = build_nc()
    nc = _NC_CACHE["nc"]
    in_maps = prep_inputs(**inputs)
    res = run_bass_kernel_spmd(nc, in_maps, core_ids=list(range(NCORES)))
    return gather_out(res.results)


if __name__ == "__main__":
    nc = build_nc()
    print("built ok")
